# revision 38
# baseline (speedup 1.0000x reference)
"""Trainium2 Bass kernel for nn_ContinuousThoughtBlock.

Strategy: pure data-parallel over batch (B=8 -> 8 NeuronCores), zero
collectives.  Each core computes one batch element end-to-end:

  context = mean_L(h)                       [D]
  ctx_n   = LN(context); th0 = ctx_n @ Wagg [D]
  8 paths evolve through 4 residual-MLP steps (bf16 matmuls, weights
  stationary on the PE, activations in a [D-on-partitions, path] layout
  so LayerNorm stats become ones-vector matmuls / partition reductions)
  amps    = pruned softmax over paths (only needed after last step)
  bc      = (sum_p amps_p * th_p) @ Wbc     [D]
  gate    = sigmoid(h @ Wg + bg)            [L, D]  (bf16, spilled to DRAM)
  out     = LN_D(h + gate * bc)             [L, D]

Key mechanics:
  - all f32 weights/activations are loaded with gpsimd casting DMAs
    (f32 DRAM -> bf16 SBUF at line rate; no staging, no cast ops)
  - h is transposed for the gate matmul with SBUF->SBUF DMA-transpose
  - gate spills to DRAM (bf16) and is re-read in the final phase
  - three DMA queues: gpsimd SWDGE = weight/h loads + gate re-read,
    ACT HWDGE = hT transposes + gate spill, SP HWDGE = small vectors +
    output writes
  - LN statistics, softmax/prune, residual adds stay in f32
"""

import numpy as np

import concourse.bass as bass
import concourse.mybir as mybir
import concourse.tile as tile
from concourse import bacc
from concourse.bass_utils import run_bass_kernel_spmd
from concourse.masks import make_identity

# Problem constants (hardcoded per harness contract).
B, L, D, H = 8, 2048, 1024, 4096
NUM_PATHS = 8
NUM_STEPS = 4
PRUNE = 0.1
EPS = 1e-6
KD = D // 128    # 8  D-chunks
KH = H // 128    # 32 H-chunks
ML = L // 128    # 16 L-tiles
INV_SQRT_D = 1.0 / float(np.sqrt(np.float32(D)))

F32 = mybir.dt.float32
BF16 = mybir.dt.bfloat16
AF = mybir.ActivationFunctionType
ALU = mybir.AluOpType
AX = mybir.AxisListType

WEIGHT_NAMES = [
    "input_norm_gamma", "input_norm_beta",
    "aggregator_weight", "aggregator_bias",
    "projector_norm_gamma", "projector_norm_beta",
    "projector_dense1_weight", "projector_dense1_bias",
    "projector_dense2_weight", "projector_dense2_bias",
    "broadcast_weight", "broadcast_bias",
    "gate_weight", "gate_bias",
    "output_norm_gamma", "output_norm_beta",
]


def _bc0(ap, n=128):
    """Broadcast a 1-D AP down n partitions via a stride-0 partition dim."""
    return bass.AP(tensor=ap.tensor, offset=ap.offset, ap=[[0, n]] + list(ap.ap))


def _rep0(ap, n, pos=1):
    """Insert a stride-0 free dim of extent n at position pos."""
    new = list(ap.ap)
    new.insert(pos, [0, n])
    return bass.AP(tensor=ap.tensor, offset=ap.offset, ap=new)


def build_graph(triv, debug=False):
    nc = bacc.Bacc("TRN2", target_bir_lowering=False, debug=False,
                   enable_asserts=True, num_devices=B)

    h_ext = nc.declare_dram_parameter("hidden_states", [L, D], BF16, isOutput=False)
    w_ext = {}
    w_ext["aggregator_weight"] = nc.declare_dram_parameter(
        "aggregator_weight", [D, D], BF16, isOutput=False)
    w_ext["projector_dense1_weight"] = nc.declare_dram_parameter(
        "projector_dense1_weight", [D, H], BF16, isOutput=False)
    w_ext["projector_dense2_weight"] = nc.declare_dram_parameter(
        "projector_dense2_weight", [H, D], BF16, isOutput=False)
    w_ext["broadcast_weight"] = nc.declare_dram_parameter(
        "broadcast_weight", [D, D], BF16, isOutput=False)
    w_ext["gate_weight"] = nc.declare_dram_parameter(
        "gate_weight", [D, D], BF16, isOutput=False)
    for n in ("input_norm_gamma", "input_norm_beta", "aggregator_bias",
              "projector_norm_gamma", "projector_norm_beta",
              "projector_dense1_bias", "projector_dense2_bias",
              "broadcast_bias", "gate_bias",
              "output_norm_gamma", "output_norm_beta"):
        shape = [H] if n == "projector_dense1_bias" else [D]
        w_ext[n] = nc.declare_dram_parameter(n, shape, F32, isOutput=False)
    out_ext = nc.declare_dram_parameter("out", [L, D], F32, isOutput=True)
    dbg = {}
    if debug:
        for nm, shape in (("d_ctxT", [128, KD]), ("d_ctxn", [128, KD]),
                          ("d_th0", [128, KD]), ("d_tT", [128, KD * NUM_PATHS]),
                          ("d_amps", [1, NUM_PATHS]), ("d_bcbf", [128, D]),
                          ("d_gate0", [128, D]), ("d_pre0", [128, D])):
            dbg[nm] = nc.declare_dram_parameter(nm, shape, F32, isOutput=True)

    with tile.TileContext(nc) as tc:
        _build_body(nc, tc, h_ext, w_ext, out_ext, triv, dbg)
    nc.compile()
    return nc


def _dmajor(nc, pool, ps_pool, ident_bf, dram_ap, n, name):
    """DMA a [n*128] DRAM vector into a [128, n] d-major SBUF tile
    (tile[p, k] = v[k*128 + p]) via a bf16 [n,128] load + PE transpose.
    Values are bf16-rounded, acceptable for gamma/beta/bias vectors."""
    rowk = pool.tile([n, 128], BF16, tag="dmaj_rowk")
    nc.gpsimd.dma_start(out=rowk[:], in_=dram_ap.rearrange("(k p) -> k p", p=128))
    ps = ps_pool.tile([128, n], BF16, tag="tr")
    nc.tensor.transpose(ps[:], rowk[:], ident_bf[0:n, 0:n])
    t = pool.tile([128, n], F32, tag=name)
    nc.scalar.copy(t[:], ps[:])
    return t


def _build_body(nc, tc, h_ext, w, out_ext, triv, dbg=None):
    dbg = dbg or {}
    import contextlib
    ctx = contextlib.ExitStack()
    with ctx:
        # ---------------- pools ----------------
        singles = ctx.enter_context(tc.tile_pool(name="singles", bufs=1))
        smalls = ctx.enter_context(tc.tile_pool(name="smalls", bufs=1))
        tstate = ctx.enter_context(tc.tile_pool(name="tstate", bufs=2))
        all_triv = all(triv.values())
        hTm_pool = ctx.enter_context(tc.tile_pool(name="hTm", bufs=2 if all_triv else 1))
        gout = ctx.enter_context(tc.tile_pool(name="gout", bufs=2 if all_triv else 1))
        rows = ctx.enter_context(tc.tile_pool(name="rows", bufs=1))
        wpool = tc.alloc_tile_pool(name="wpool", bufs=1)
        dram = ctx.enter_context(tc.tile_pool(name="dram", bufs=1, space="DRAM"))

        ps_small = ctx.enter_context(tc.tile_pool(name="ps_small", bufs=1, space="PSUM"))
        ps_tr = ctx.enter_context(tc.tile_pool(name="ps_tr", bufs=1, space="PSUM"))
        ps_gate = ctx.enter_context(tc.tile_pool(name="ps_gate", bufs=2, space="PSUM"))
        ps_th = ctx.enter_context(tc.tile_pool(name="ps_th", bufs=2, space="PSUM"))

        # ---------------- constants ----------------
        ident_bf = singles.tile([128, 128], BF16)
        make_identity(nc, ident_bf[:])
        ones_bf = singles.tile([128, 1], BF16)
        nc.vector.memset(ones_bf[:], 1.0)
        ones_f32 = singles.tile([128, 1], F32)
        nc.vector.memset(ones_f32[:], 1.0)
        ones_row = singles.tile([1, 128], F32)
        nc.vector.memset(ones_row[:], 1.0)
        ones_row_bf = singles.tile([1, 128], BF16)
        nc.vector.memset(ones_row_bf[:], 1.0)
        eps1 = singles.tile([1, 1], F32)
        nc.vector.memset(eps1[:], EPS)
        eps_col = singles.tile([128, 1], F32)
        nc.vector.memset(eps_col[:], EPS)

        # resident (bf16) tensors
        h_bf = singles.tile([128, ML, D], BF16)      # 32KB/part
        w1_bf = wpool.tile([128, KD, H], BF16)       # 64KB/part
        w2_bf = wpool.tile([128, KH, D], BF16)       # 64KB/part
        wg_bf = wpool.tile([128, KD, D], BF16)       # 16KB/part
        wab_bf = wpool.tile([128, KD, D], BF16)      # 16KB/part (Wagg, later Wbc)

        gate_dram = dram.tile([L, D], BF16)

        # d-major vectors (only when nontrivial)
        gammaT_in = betaT_in = None
        if not triv["input_norm"]:
            gammaT_in = _dmajor(nc, singles, ps_tr, ident_bf,
                                w["input_norm_gamma"].ap(), KD, "g_in")
            betaT_in = _dmajor(nc, singles, ps_tr, ident_bf,
                               w["input_norm_beta"].ap(), KD, "b_in")
        gammaT_pr = betaT_pr = None
        if not triv["projector_norm"]:
            gammaT_pr = _dmajor(nc, singles, ps_tr, ident_bf,
                                w["projector_norm_gamma"].ap(), KD, "g_pr")
            betaT_pr = _dmajor(nc, singles, ps_tr, ident_bf,
                               w["projector_norm_beta"].ap(), KD, "b_pr")
        baggT = None
        if not triv["aggregator_bias"]:
            baggT = _dmajor(nc, singles, ps_tr, ident_bf,
                            w["aggregator_bias"].ap(), KD, "bagg")
        b1T = None
        if not triv["projector_dense1_bias"]:
            b1T = _dmajor(nc, singles, ps_tr, ident_bf,
                          w["projector_dense1_bias"].ap(), KH, "b1")
        b2T_rep = None
        if not triv["projector_dense2_bias"]:
            b2T = _dmajor(nc, singles, ps_tr, ident_bf,
                          w["projector_dense2_bias"].ap(), KD, "b2")
            b2T_rep = _rep0(b2T[:], NUM_PATHS, pos=2)  # [128, KD, P] view
        gbias_row = None
        if not triv["gate_bias"]:
            gbias_row = rows.tile([1, D], BF16, tag="brow")
            nc.gpsimd.dma_start(out=gbias_row[:],
                                in_=w["gate_bias"].ap().rearrange("(a d) -> a d", a=1))

        # ---------------- phase 1: load h (casting DMA) + context ----------------
        h_src = h_ext.ap().rearrange("(m t p) d -> p m t d", p=128, t=2)
        ctx_ps = ps_small.tile([128, KD], F32, tag="sm")
        for m2 in range(ML // 2):
            nc.sync.dma_start(out=h_bf[:, 2 * m2:2 * m2 + 2, :], in_=h_src[:, m2])
        for k in range(KD):
            for m in range(ML):
                nc.tensor.matmul(ctx_ps[:, k:k + 1],
                                 h_bf[:, m, k * 128:(k + 1) * 128],
                                 ones_bf[:],
                                 start=(m == 0), stop=(m == ML - 1))
        # ctxT[p, k] = context[k*128+p] = mean over L
        ctxT = singles.tile([128, KD], F32)
        nc.scalar.mul(ctxT[:], ctx_ps[:], 1.0 / L)
        if "d_ctxT" in dbg:
            nc.sync.dma_start(out=dbg["d_ctxT"].ap(), in_=ctxT[:])

        # ---------------- phase 2a: Wg (casting DMA) ----------------
        wg_src = w["gate_weight"].ap().rearrange("(k t p) d -> p k t d", p=128, t=2)
        for k2 in range(KD // 2):
            nc.sync.dma_start(out=wg_bf[:, 2 * k2:2 * k2 + 2, :], in_=wg_src[:, k2])

        # ---------------- phase 3: input LN + thought0 ----------------
        sqc = smalls.tile([128, KD], F32, tag="sqc")
        nc.vector.tensor_mul(sqc[:], ctxT[:], ctxT[:])
        cst_ps = ps_small.tile([1, 2 * KD], F32, tag="sm")
        nc.tensor.matmul(cst_ps[0:1, 0:KD], ones_f32[:], ctxT[:], start=True, stop=True)
        nc.tensor.matmul(cst_ps[0:1, KD:2 * KD], ones_f32[:], sqc[:], start=True, stop=True)
        csums = smalls.tile([1, 2], F32, tag="csums")
        nc.vector.tensor_reduce(csums[0:1, 0:1], cst_ps[0:1, 0:KD], axis=AX.X, op=ALU.add)
        nc.vector.tensor_reduce(csums[0:1, 1:2], cst_ps[0:1, KD:2 * KD], axis=AX.X, op=ALU.add)
        cmr = smalls.tile([1, 2], F32, tag="cmr")      # [mean, rstd]
        nc.scalar.mul(cmr[0:1, 0:1], csums[0:1, 0:1], 1.0 / D)
        csq = smalls.tile([1, 2], F32, tag="csq")
        nc.scalar.mul(csq[0:1, 0:1], csums[0:1, 1:2], 1.0 / D)   # E[x^2]
        nc.vector.tensor_mul(csq[0:1, 1:2], cmr[0:1, 0:1], cmr[0:1, 0:1])  # mean^2
        cvar = smalls.tile([1, 1], F32, tag="cvar")
        nc.vector.tensor_sub(cvar[:], csq[0:1, 0:1], csq[0:1, 1:2])
        nc.scalar.activation(cvar[:], cvar[:], AF.Sqrt, bias=eps1[0:1, :])
        nc.vector.reciprocal(cmr[0:1, 1:2], cvar[:])
        cmr_ps = ps_small.tile([128, 2], F32, tag="sm")
        nc.tensor.matmul(cmr_ps[:], ones_row[:], cmr[:], start=True, stop=True)
        cmr_b = smalls.tile([128, 2], F32, tag="cmrb")
        nc.scalar.copy(cmr_b[:], cmr_ps[:])
        ctxn = smalls.tile([128, KD], F32, tag="ctxn")
        nc.vector.tensor_scalar(ctxn[:], ctxT[:], cmr_b[:, 0:1], cmr_b[:, 1:2],
                                op0=ALU.subtract, op1=ALU.mult)
        if gammaT_in is not None:
            nc.vector.tensor_mul(ctxn[:], ctxn[:], gammaT_in[:])
            nc.vector.tensor_add(ctxn[:], ctxn[:], betaT_in[:])
        if "d_ctxn" in dbg:
            nc.sync.dma_start(out=dbg["d_ctxn"].ap(), in_=ctxn[:])
        ctxn_bf = smalls.tile([128, KD], BF16, tag="ctxnbf")
        nc.vector.tensor_copy(ctxn_bf[:], ctxn[:])

        # Wagg (casting DMA into the shared wab buffer)
        wagg_src = w["aggregator_weight"].ap().rearrange("(k t p) d -> p k t d",
                                                         p=128, t=2)
        for k2 in range(KD // 2):
            nc.sync.dma_start(out=wab_bf[:, 2 * k2:2 * k2 + 2, :],
                              in_=wagg_src[:, k2])
        # thought0 = ctx_n @ Wagg, d-major via per-k single-instr psum groups
        th0acc = smalls.tile([128, KD], F32, tag="th0acc")
        for k in range(KD):
            thp = ps_small.tile([128, KD], F32, tag="sm")
            for dm in range(KD):
                nc.tensor.matmul(thp[:, dm:dm + 1],
                                 wab_bf[:, k, dm * 128:(dm + 1) * 128],
                                 ctxn_bf[:, k:k + 1], start=True, stop=True)
            if k == 0:
                nc.vector.tensor_copy(th0acc[:], thp[:])
            else:
                nc.vector.tensor_add(th0acc[:], th0acc[:], thp[:])
        if baggT is not None:
            nc.vector.tensor_add(th0acc[:], th0acc[:], baggT[:])
        if "d_th0" in dbg:
            nc.sync.dma_start(out=dbg["d_th0"].ap(), in_=th0acc[:])
        # seed 8 paths: tT[p, k, q] = th0[k*128+p] * (1 + 0.02 q)
        tT = tstate.tile([128, KD, NUM_PATHS], F32, tag="tT")
        for q in range(NUM_PATHS):
            nc.scalar.mul(tT[:, :, q], th0acc[:], 1.0 + 0.02 * q)

        # ---------------- phase 7: gate matmul (spilled to DRAM) ----------------
        # hT via SBUF->SBUF DMA transpose; lhsT = hT chunks, rhs = Wg.
        for m in range(ML):
            hTm = hTm_pool.tile([128, KD, 128], BF16, tag="hTm")
            nc.scalar.dma_start_transpose(out=hTm[:], in_=h_bf[:, m, :])
            for n in range(2):
                g_ps = ps_gate.tile([128, 512], F32, tag="gps")
                for k in range(KD):
                    nc.tensor.matmul(g_ps[:], hTm[:, k, :],
                                     wg_bf[:, k, n * 512:(n + 1) * 512],
                                     start=(k == 0),
                                     stop=(k == KD - 1 and gbias_row is None))
                if gbias_row is not None:
                    nc.tensor.matmul(g_ps[:], ones_row_bf[:],
                                     gbias_row[0:1, n * 512:(n + 1) * 512],
                                     start=False, stop=True)
                g_sb = gout.tile([128, 512], BF16, tag="gout")
                nc.scalar.activation(g_sb[:], g_ps[:], AF.Sigmoid)
                nc.scalar.dma_start(
                    out=gate_dram[m * 128:(m + 1) * 128, n * 512:(n + 1) * 512],
                    in_=g_sb[:])

        # ---------------- phase 2c/2d: W1 / W2 (casting DMAs) ----------------
        w1_src = w["projector_dense1_weight"].ap().rearrange("(k p) h -> p k h", p=128)
        for k in range(KD):
            nc.sync.dma_start(out=w1_bf[:, k, :], in_=w1_src[:, k])
        w2_src = w["projector_dense2_weight"].ap().rearrange("(k t p) d -> p k t d",
                                                             p=128, t=4)
        for k4 in range(KH // 4):
            nc.sync.dma_start(out=w2_bf[:, 4 * k4:4 * k4 + 4, :], in_=w2_src[:, k4])

        # ---------------- phase 4: thought steps ----------------
        for step in range(NUM_STEPS):
            last = step == NUM_STEPS - 1
            sq = smalls.tile([128, KD, NUM_PATHS], F32, tag="sq")
            nc.vector.tensor_mul(sq[:], tT[:], tT[:])
            st_ps = ps_small.tile([1, 128], F32, tag="sm")
            nc.tensor.matmul(st_ps[0:1, 0:64], ones_f32[:],
                             tT[:].rearrange("a k q -> a q k"), start=True, stop=True)
            nc.tensor.matmul(st_ps[0:1, 64:128], ones_f32[:],
                             sq[:].rearrange("a k q -> a q k"), start=True, stop=True)
            sums = smalls.tile([1, 2 * NUM_PATHS], F32, tag="sums")
            nc.vector.tensor_reduce(sums[0:1, 0:NUM_PATHS],
                                    st_ps[0:1, 0:64].rearrange("a (q k) -> a q k", k=KD),
                                    axis=AX.X, op=ALU.add)
            nc.vector.tensor_reduce(sums[0:1, NUM_PATHS:],
                                    st_ps[0:1, 64:128].rearrange("a (q k) -> a q k", k=KD),
                                    axis=AX.X, op=ALU.add)
            mr = smalls.tile([1, 2 * NUM_PATHS], F32, tag="mr")  # [mean(8), rstd(8)]
            nc.scalar.mul(mr[0:1, 0:NUM_PATHS], sums[0:1, 0:NUM_PATHS], 1.0 / D)
            msq = smalls.tile([1, NUM_PATHS], F32, tag="msq")
            nc.scalar.mul(msq[0:1, :], sums[0:1, NUM_PATHS:], 1.0 / D)
            m2 = smalls.tile([1, NUM_PATHS], F32, tag="m2")
            nc.vector.tensor_mul(m2[0:1, :], mr[0:1, 0:NUM_PATHS], mr[0:1, 0:NUM_PATHS])
            var = smalls.tile([1, NUM_PATHS], F32, tag="var")
            nc.vector.tensor_sub(var[0:1, :], msq[0:1, :], m2[0:1, :])
            nc.scalar.activation(var[0:1, :], var[0:1, :], AF.Sqrt, bias=eps1[0:1, :])
            nc.vector.reciprocal(mr[0:1, NUM_PATHS:], var[0:1, :])
            mr_ps = ps_small.tile([128, 2 * NUM_PATHS], F32, tag="sm")
            nc.tensor.matmul(mr_ps[:], ones_row[:], mr[:], start=True, stop=True)
            mr_b = smalls.tile([128, 2 * NUM_PATHS], F32, tag="mrb")
            nc.scalar.copy(mr_b[:], mr_ps[:])

            # normalize all (k, q) at once with stride-0 broadcasts
            tn_bf = smalls.tile([128, KD, NUM_PATHS], BF16, tag="tnbf")
            tc_f = smalls.tile([128, KD, NUM_PATHS], F32, tag="tcf")
            nc.vector.tensor_tensor(out=tc_f[:], in0=tT[:],
                                    in1=_rep0(mr_b[:, 0:NUM_PATHS], KD),
                                    op=ALU.subtract)
            if gammaT_pr is not None:
                nc.vector.tensor_tensor(out=tc_f[:], in0=tc_f[:],
                                        in1=_rep0(mr_b[:, NUM_PATHS:], KD),
                                        op=ALU.mult)
                nc.vector.tensor_tensor(out=tc_f[:], in0=tc_f[:],
                                        in1=_rep0(gammaT_pr[:], NUM_PATHS, pos=2),
                                        op=ALU.mult)
                nc.vector.tensor_tensor(out=tn_bf[:], in0=tc_f[:],
                                        in1=_rep0(betaT_pr[:], NUM_PATHS, pos=2),
                                        op=ALU.add)
            else:
                nc.vector.tensor_tensor(out=tn_bf[:], in0=tc_f[:],
                                        in1=_rep0(mr_b[:, NUM_PATHS:], KD),
                                        op=ALU.mult)

            # dense1: x1 = gelu(tn @ W1 [+ b1]) in [H-part, path] layout
            x1_bf = smalls.tile([128, KH // 8, 8, NUM_PATHS], BF16, tag="x1")
            for tblk in range(KH // 8):
                x1_ps = ps_th.tile([128, 8 * NUM_PATHS], F32, tag="th")
                for hs in range(8):
                    mh = tblk * 8 + hs
                    for k in range(KD):
                        nc.tensor.matmul(x1_ps[:, hs * 8:(hs + 1) * 8],
                                         w1_bf[:, k, mh * 128:(mh + 1) * 128],
                                         tn_bf[:, k, :],
                                         start=(k == 0), stop=(k == KD - 1))
                # tanh-gelu (matches jax.nn.gelu approximate=True)
                xs = smalls.tile([128, 8 * NUM_PATHS], F32, tag="gelu_x")
                if b1T is not None:
                    for hs in range(8):
                        mh = tblk * 8 + hs
                        nc.scalar.activation(xs[:, hs * 8:(hs + 1) * 8],
                                             x1_ps[:, hs * 8:(hs + 1) * 8],
                                             AF.Identity, bias=b1T[:, mh:mh + 1])
                else:
                    nc.scalar.copy(xs[:], x1_ps[:])
                u = smalls.tile([128, 8 * NUM_PATHS], F32, tag="gelu_u")
                nc.vector.tensor_mul(u[:], xs[:], xs[:])
                nc.vector.tensor_mul(u[:], u[:], xs[:])
                nc.vector.scalar_tensor_tensor(u[:], u[:], 0.044715, xs[:],
                                               op0=ALU.mult, op1=ALU.add)
                nc.scalar.activation(u[:], u[:], AF.Tanh, scale=0.7978845608028654)
                nc.vector.scalar_tensor_tensor(u[:], u[:], 1.0, xs[:],
                                               op0=ALU.add, op1=ALU.mult)
                nc.scalar.mul(x1_bf[:, tblk].rearrange("a b c -> a (b c)"), u[:], 0.5)

            # dense2 + residual
            y_ps = ps_th.tile([128, KD * NUM_PATHS], F32, tag="th")
            for dm in range(KD):
                for hk in range(KH):
                    nc.tensor.matmul(y_ps[:, dm * 8:(dm + 1) * 8],
                                     w2_bf[:, hk, dm * 128:(dm + 1) * 128],
                                     x1_bf[:, hk // 8, hk % 8, :],
                                     start=(hk == 0), stop=(hk == KH - 1))
            tT_new = tstate.tile([128, KD, NUM_PATHS], F32, tag="tT")
            yv = y_ps[:].rearrange("a (k q) -> a k q", k=KD)
            if b2T_rep is not None:
                nc.vector.tensor_add(tT_new[:], yv, b2T_rep)
                nc.vector.tensor_add(tT_new[:], tT_new[:], tT[:])
            else:
                nc.vector.tensor_add(tT_new[:], yv, tT[:])
            tT = tT_new

            if last:
                sc_ps = ps_small.tile([1, NUM_PATHS], F32, tag="sm")
                for k in range(KD):
                    nc.tensor.matmul(sc_ps[:], ctxT[:, k:k + 1], tT[:, k, :],
                                     start=(k == 0), stop=(k == KD - 1))
                sc = smalls.tile([1, NUM_PATHS], F32, tag="sc")
                nc.scalar.mul(sc[:], sc_ps[:], INV_SQRT_D)
                negmax = smalls.tile([1, 1], F32, tag="negmax")
                nc.vector.tensor_reduce(negmax[:], sc[:], axis=AX.X, op=ALU.max,
                                        negate=True)
                esum = smalls.tile([1, 1], F32, tag="esum")
                ex = smalls.tile([1, NUM_PATHS], F32, tag="ex")
                nc.scalar.activation(ex[:], sc[:], AF.Exp, bias=negmax[0:1, :],
                                     accum_out=esum[:])
                rsum = smalls.tile([1, 1], F32, tag="rsum")
                nc.vector.reciprocal(rsum[:], esum[:])
                amps0 = smalls.tile([1, NUM_PATHS], F32, tag="amps0")
                nc.vector.tensor_scalar(amps0[:], ex[:], rsum[0:1, :], None, op0=ALU.mult)
                mask = smalls.tile([1, NUM_PATHS], F32, tag="mask")
                nc.vector.tensor_scalar(mask[:], amps0[:], PRUNE, None, op0=ALU.is_ge)
                pruned = smalls.tile([1, NUM_PATHS], F32, tag="pruned")
                nc.vector.tensor_mul(pruned[:], amps0[:], mask[:])
                psum_s = smalls.tile([1, 1], F32, tag="psums")
                nc.vector.tensor_reduce(psum_s[:], pruned[:], axis=AX.X, op=ALU.add)
                nc.vector.tensor_scalar(psum_s[:], psum_s[:], EPS, None, op0=ALU.add)
                rr = smalls.tile([1, 1], F32, tag="rr")
                nc.vector.reciprocal(rr[:], psum_s[:])
                ampsF = smalls.tile([1, NUM_PATHS], F32, tag="ampsF")
                nc.vector.tensor_scalar(ampsF[:], pruned[:], rr[0:1, :], None, op0=ALU.mult)

        if "d_tT" in dbg:
            nc.sync.dma_start(out=dbg["d_tT"].ap(),
                              in_=tT[:].rearrange("a k q -> a (k q)"))
        if "d_amps" in dbg:
            nc.sync.dma_start(out=dbg["d_amps"].ap(), in_=ampsF[:])

        # ---------------- phase 5: collapse + bc ----------------
        ab_ps = ps_small.tile([128, NUM_PATHS], F32, tag="sm")
        nc.tensor.matmul(ab_ps[:], ones_row[:], ampsF[:], start=True, stop=True)
        amps_sb = smalls.tile([128, NUM_PATHS], F32, tag="ampssb")
        nc.scalar.copy(amps_sb[:], ab_ps[:])
        prod = smalls.tile([128, KD, NUM_PATHS], F32, tag="prod")
        nc.vector.tensor_tensor(out=prod[:], in0=tT[:], in1=_rep0(amps_sb[:], KD),
                                op=ALU.mult)
        finalT = smalls.tile([128, KD], F32, tag="finalT")
        nc.vector.tensor_reduce(finalT[:], prod[:], axis=AX.X, op=ALU.add)
        finalT_bf = smalls.tile([128, KD], BF16, tag="finalTbf")
        nc.vector.tensor_copy(finalT_bf[:], finalT[:])

        # Wbc overwrites the shared wab buffer (casting DMA)
        wbc_src = w["broadcast_weight"].ap().rearrange("(k t p) d -> p k t d",
                                                       p=128, t=2)
        for k2 in range(KD // 2):
            nc.sync.dma_start(out=wab_bf[:, 2 * k2:2 * k2 + 2, :],
                              in_=wbc_src[:, k2])
        # bc row [1, D] in two 512-halves, then K=1 matmul broadcast
        bc_bf = singles.tile([128, D], BF16)
        for n in range(2):
            bc_ps = ps_gate.tile([1, 512], F32, tag="gps")
            for k in range(KD):
                nc.tensor.matmul(bc_ps[:],
                                 finalT_bf[:, k:k + 1],
                                 wab_bf[:, k, n * 512:(n + 1) * 512],
                                 start=(k == 0), stop=(k == KD - 1))
            bc_half = rows.tile([1, 512], F32, tag="row")
            if not triv["broadcast_bias"]:
                bbh = rows.tile([1, 512], F32, tag="brow")
                nc.sync.dma_start(
                    out=bbh[:],
                    in_=w["broadcast_bias"].ap()[n * 512:(n + 1) * 512]
                        .rearrange("(a d) -> a d", a=1))
                nc.vector.tensor_add(bc_half[:], bc_ps[:], bbh[:])
            else:
                nc.scalar.copy(bc_half[:], bc_ps[:])
            bcb_ps = ps_gate.tile([128, 512], F32, tag="gps")
            nc.tensor.matmul(bcb_ps[:], ones_row[:], bc_half[:], start=True, stop=True)
            nc.scalar.copy(bc_bf[:, n * 512:(n + 1) * 512], bcb_ps[:])
        if "d_bcbf" in dbg:
            nc.sync.dma_start(out=dbg["d_bcbf"].ap(), in_=bc_bf[:])

        # release the weight pool; final-phase pools reuse the space
        wpool.release()
        gin = ctx.enter_context(tc.tile_pool(name="gin", bufs=3))
        fin = ctx.enter_context(tc.tile_pool(name="fin", bufs=3))
        gamma_out_b = beta_out_b = None
        if not triv["output_norm"]:
            fin1 = ctx.enter_context(tc.tile_pool(name="fin1", bufs=1))
            gamma_out_b = fin1.tile([128, D], F32)
            nc.sync.dma_start(out=gamma_out_b[:], in_=_bc0(w["output_norm_gamma"].ap()))
            beta_out_b = fin1.tile([128, D], F32)
            nc.sync.dma_start(out=beta_out_b[:], in_=_bc0(w["output_norm_beta"].ap()))

        # ---------------- phase 8: final LN + output ----------------
        # all elementwise work in bf16 (DVE 4x mode); stats accumulate in f32
        for m in range(ML):
            g_in = gin.tile([128, D], BF16, tag="gin")
            nc.gpsimd.dma_start(out=g_in[:], in_=gate_dram[m * 128:(m + 1) * 128, :])
            if m == 0 and "d_gate0" in dbg:
                nc.sync.dma_start(out=dbg["d_gate0"].ap(), in_=g_in[:])
            p1 = gin.tile([128, D], BF16, tag="p1")
            nc.vector.tensor_mul(p1[:], g_in[:], bc_bf[:])
            pre = fin.tile([128, D], BF16, tag="pre")
            nc.vector.tensor_add(pre[:], p1[:], h_bf[:, m, :])
            if m == 0 and "d_pre0" in dbg:
                nc.sync.dma_start(out=dbg["d_pre0"].ap(), in_=pre[:])
            stats = fin.tile([128, 2, 6], F32, tag="stats")
            pre_v = pre[:].rearrange("a (s x) -> a s x", s=2)
            for s in range(2):
                nc.vector.bn_stats(stats[:, s, :], pre_v[:, s, :])
            mv = fin.tile([128, 2], F32, tag="mv")
            nc.vector.bn_aggr(mv[:], stats[:])
            sd = fin.tile([128, 1], F32, tag="sd")
            nc.scalar.activation(sd[:], mv[:, 1:2], AF.Sqrt, bias=eps_col[:])
            rstd = fin.tile([128, 1], F32, tag="rstd")
            nc.vector.reciprocal(rstd[:], sd[:])
            nmr = fin.tile([128, 1], F32, tag="nmr")
            nc.vector.tensor_scalar(nmr[:], mv[:, 0:1], rstd[:, 0:1], -1.0,
                                    op0=ALU.mult, op1=ALU.mult)
            o = fin.tile([128, D], F32, tag="o")
            nc.scalar.activation(o[:], pre[:], AF.Identity,
                                 bias=nmr[:, 0:1], scale=rstd[:, 0:1])
            if gamma_out_b is not None:
                nc.vector.tensor_mul(o[:], o[:], gamma_out_b[:])
                nc.vector.tensor_add(o[:], o[:], beta_out_b[:])
            nc.sync.dma_start(out=out_ext.ap()[m * 128:(m + 1) * 128, :], in_=o[:])


def _triv_flags(inputs):
    def ones(x):
        return bool(np.all(np.asarray(x) == 1.0))

    def zeros(x):
        return bool(np.all(np.asarray(x) == 0.0))

    return {
        "input_norm": ones(inputs["input_norm_gamma"]) and zeros(inputs["input_norm_beta"]),
        "projector_norm": ones(inputs["projector_norm_gamma"]) and zeros(inputs["projector_norm_beta"]),
        "output_norm": ones(inputs["output_norm_gamma"]) and zeros(inputs["output_norm_beta"]),
        "aggregator_bias": zeros(inputs["aggregator_bias"]),
        "projector_dense1_bias": zeros(inputs["projector_dense1_bias"]),
        "projector_dense2_bias": zeros(inputs["projector_dense2_bias"]),
        "broadcast_bias": zeros(inputs["broadcast_bias"]),
        "gate_bias": zeros(inputs["gate_bias"]),
    }


_GRAPH_CACHE = {}

BF16_INPUTS = ("hidden_states", "aggregator_weight", "projector_dense1_weight",
               "projector_dense2_weight", "broadcast_weight", "gate_weight")


def prep_in_maps(inputs):
    """Build per-core input maps; big tensors are converted to bf16 on the
    host (round-to-nearest) so the NEFF reads half the bytes."""
    import ml_dtypes
    hs = np.ascontiguousarray(
        np.asarray(inputs["hidden_states"], dtype=np.float32).astype(ml_dtypes.bfloat16))
    assert hs.shape == (B, L, D)
    weights = {}
    for n in WEIGHT_NAMES:
        a = np.asarray(inputs[n], dtype=np.float32)
        if n in BF16_INPUTS:
            a = a.astype(ml_dtypes.bfloat16)
        weights[n] = np.ascontiguousarray(a)
    in_maps = []
    for b in range(B):
        m = {"hidden_states": np.ascontiguousarray(hs[b])}
        m.update(weights)
        in_maps.append(m)
    return in_maps


def kernel(**inputs):
    triv = _triv_flags(inputs)
    key = tuple(sorted(triv.items()))
    if key not in _GRAPH_CACHE:
        _GRAPH_CACHE[key] = build_graph(triv)
    nc = _GRAPH_CACHE[key]
    in_maps = prep_in_maps(inputs)
    res = run_bass_kernel_spmd(nc, in_maps, core_ids=list(range(B)))
    out = np.stack([res.results[b]["out"] for b in range(B)], axis=0)
    return out.astype(np.float32)


# revision 40
# speedup vs baseline: 1.1061x; 1.1061x over previous
"""Trainium2 Bass kernel for nn_ContinuousThoughtBlock.

Strategy: pure data-parallel over batch (B=8 -> 8 NeuronCores), zero
collectives.  Each core computes one batch element end-to-end:

  context = mean_L(h)                       [D]
  ctx_n   = LN(context); th0 = ctx_n @ Wagg [D]
  8 paths evolve through 4 residual-MLP steps (bf16 matmuls, weights
  stationary on the PE, activations in a [D-on-partitions, path] layout
  so LayerNorm stats become ones-vector matmuls / partition reductions)
  amps    = pruned softmax over paths (only needed after last step)
  bc      = (sum_p amps_p * th_p) @ Wbc     [D]
  gate    = sigmoid(h @ Wg + bg)            [L, D]  (bf16, spilled to DRAM)
  out     = LN_D(h + gate * bc)             [L, D]

Key mechanics:
  - all f32 weights/activations are loaded with gpsimd casting DMAs
    (f32 DRAM -> bf16 SBUF at line rate; no staging, no cast ops)
  - h is transposed for the gate matmul with SBUF->SBUF DMA-transpose
  - gate spills to DRAM (bf16) and is re-read in the final phase
  - three DMA queues: gpsimd SWDGE = weight/h loads + gate re-read,
    ACT HWDGE = hT transposes + gate spill, SP HWDGE = small vectors +
    output writes
  - LN statistics, softmax/prune, residual adds stay in f32
"""

import numpy as np

import concourse.bass as bass
import concourse.mybir as mybir
import concourse.tile as tile
from concourse import bacc
from concourse.bass_utils import run_bass_kernel_spmd
from concourse.masks import make_identity

# Problem constants (hardcoded per harness contract).
B, L, D, H = 8, 2048, 1024, 4096
NUM_PATHS = 8
NUM_STEPS = 4
PRUNE = 0.1
EPS = 1e-6
KD = D // 128    # 8  D-chunks
KH = H // 128    # 32 H-chunks
ML = L // 128    # 16 L-tiles
INV_SQRT_D = 1.0 / float(np.sqrt(np.float32(D)))

F32 = mybir.dt.float32
BF16 = mybir.dt.bfloat16
AF = mybir.ActivationFunctionType
ALU = mybir.AluOpType
AX = mybir.AxisListType

WEIGHT_NAMES = [
    "input_norm_gamma", "input_norm_beta",
    "aggregator_weight", "aggregator_bias",
    "projector_norm_gamma", "projector_norm_beta",
    "projector_dense1_weight", "projector_dense1_bias",
    "projector_dense2_weight", "projector_dense2_bias",
    "broadcast_weight", "broadcast_bias",
    "gate_weight", "gate_bias",
    "output_norm_gamma", "output_norm_beta",
]


def _bc0(ap, n=128):
    """Broadcast a 1-D AP down n partitions via a stride-0 partition dim."""
    return bass.AP(tensor=ap.tensor, offset=ap.offset, ap=[[0, n]] + list(ap.ap))


def _rep0(ap, n, pos=1):
    """Insert a stride-0 free dim of extent n at position pos."""
    new = list(ap.ap)
    new.insert(pos, [0, n])
    return bass.AP(tensor=ap.tensor, offset=ap.offset, ap=new)


def build_graph(triv, debug=False):
    nc = bacc.Bacc("TRN2", target_bir_lowering=False, debug=False,
                   enable_asserts=True, num_devices=B)

    h_ext = nc.declare_dram_parameter("hidden_states", [L, D], BF16, isOutput=False)
    w_ext = {}
    w_ext["aggregator_weight"] = nc.declare_dram_parameter(
        "aggregator_weight", [D, D], BF16, isOutput=False)
    w_ext["projector_dense1_weight"] = nc.declare_dram_parameter(
        "projector_dense1_weight", [D, H], BF16, isOutput=False)
    w_ext["projector_dense2_weight"] = nc.declare_dram_parameter(
        "projector_dense2_weight", [H, D], BF16, isOutput=False)
    w_ext["broadcast_weight"] = nc.declare_dram_parameter(
        "broadcast_weight", [D, D], BF16, isOutput=False)
    w_ext["gate_weight"] = nc.declare_dram_parameter(
        "gate_weight", [D, D], BF16, isOutput=False)
    for n in ("input_norm_gamma", "input_norm_beta", "aggregator_bias",
              "projector_norm_gamma", "projector_norm_beta",
              "projector_dense1_bias", "projector_dense2_bias",
              "broadcast_bias", "gate_bias",
              "output_norm_gamma", "output_norm_beta"):
        shape = [H] if n == "projector_dense1_bias" else [D]
        w_ext[n] = nc.declare_dram_parameter(n, shape, F32, isOutput=False)
    out_ext = nc.declare_dram_parameter("out", [L, D], F32, isOutput=True)
    dbg = {}
    if debug:
        for nm, shape in (("d_ctxT", [128, KD]), ("d_ctxn", [128, KD]),
                          ("d_th0", [128, KD]), ("d_tT", [128, KD * NUM_PATHS]),
                          ("d_amps", [1, NUM_PATHS]), ("d_bcbf", [128, D]),
                          ("d_gate0", [128, D]), ("d_pre0", [128, D])):
            dbg[nm] = nc.declare_dram_parameter(nm, shape, F32, isOutput=True)

    with tile.TileContext(nc) as tc:
        _build_body(nc, tc, h_ext, w_ext, out_ext, triv, dbg)
    nc.compile()
    return nc


def _dmajor(nc, pool, ps_pool, ident_bf, dram_ap, n, name):
    """DMA a [n*128] DRAM vector into a [128, n] d-major SBUF tile
    (tile[p, k] = v[k*128 + p]) via a bf16 [n,128] load + PE transpose.
    Values are bf16-rounded, acceptable for gamma/beta/bias vectors."""
    rowk = pool.tile([n, 128], BF16, tag="dmaj_rowk")
    nc.gpsimd.dma_start(out=rowk[:], in_=dram_ap.rearrange("(k p) -> k p", p=128))
    ps = ps_pool.tile([128, n], BF16, tag="tr")
    nc.tensor.transpose(ps[:], rowk[:], ident_bf[0:n, 0:n])
    t = pool.tile([128, n], F32, tag=name)
    nc.scalar.copy(t[:], ps[:])
    return t


def _build_body(nc, tc, h_ext, w, out_ext, triv, dbg=None):
    dbg = dbg or {}
    import contextlib
    ctx = contextlib.ExitStack()
    with ctx:
        # ---------------- pools ----------------
        singles = ctx.enter_context(tc.tile_pool(name="singles", bufs=1))
        smalls = ctx.enter_context(tc.tile_pool(name="smalls", bufs=1))
        tstate = ctx.enter_context(tc.tile_pool(name="tstate", bufs=2))
        all_triv = all(triv.values())
        hTm_pool = ctx.enter_context(tc.tile_pool(name="hTm", bufs=2 if all_triv else 1))
        gout = ctx.enter_context(tc.tile_pool(name="gout", bufs=2 if all_triv else 1))
        rows = ctx.enter_context(tc.tile_pool(name="rows", bufs=1))
        wpool = tc.alloc_tile_pool(name="wpool", bufs=1)
        dram = ctx.enter_context(tc.tile_pool(name="dram", bufs=1, space="DRAM"))

        ps_small = ctx.enter_context(tc.tile_pool(name="ps_small", bufs=1, space="PSUM"))
        ps_tr = ctx.enter_context(tc.tile_pool(name="ps_tr", bufs=1, space="PSUM"))
        ps_gate = ctx.enter_context(tc.tile_pool(name="ps_gate", bufs=2, space="PSUM"))
        ps_th = ctx.enter_context(tc.tile_pool(name="ps_th", bufs=2, space="PSUM"))

        # ---------------- constants ----------------
        ident_bf = singles.tile([128, 128], BF16)
        make_identity(nc, ident_bf[:])
        ones_bf = singles.tile([128, 1], BF16)
        nc.vector.memset(ones_bf[:], 1.0)
        ones_f32 = singles.tile([128, 1], F32)
        nc.vector.memset(ones_f32[:], 1.0)
        ones_row = singles.tile([1, 128], F32)
        nc.vector.memset(ones_row[:], 1.0)
        ones_row_bf = singles.tile([1, 128], BF16)
        nc.vector.memset(ones_row_bf[:], 1.0)
        eps1 = singles.tile([1, 1], F32)
        nc.vector.memset(eps1[:], EPS)
        eps_col = singles.tile([128, 1], F32)
        nc.vector.memset(eps_col[:], EPS)

        # resident (bf16) tensors
        h_bf = singles.tile([128, ML, D], BF16)      # 32KB/part
        w1_bf = wpool.tile([128, KD, H], BF16)       # 64KB/part
        w2_bf = wpool.tile([128, KH, D], BF16)       # 64KB/part
        wg_bf = wpool.tile([128, KD, D], BF16)       # 16KB/part
        wab_bf = wpool.tile([128, KD, D], BF16)      # 16KB/part (Wagg, later Wbc)

        gate_dram = dram.tile([L, D], BF16)

        # d-major vectors (only when nontrivial)
        gammaT_in = betaT_in = None
        if not triv["input_norm"]:
            gammaT_in = _dmajor(nc, singles, ps_tr, ident_bf,
                                w["input_norm_gamma"].ap(), KD, "g_in")
            betaT_in = _dmajor(nc, singles, ps_tr, ident_bf,
                               w["input_norm_beta"].ap(), KD, "b_in")
        gammaT_pr = betaT_pr = None
        if not triv["projector_norm"]:
            gammaT_pr = _dmajor(nc, singles, ps_tr, ident_bf,
                                w["projector_norm_gamma"].ap(), KD, "g_pr")
            betaT_pr = _dmajor(nc, singles, ps_tr, ident_bf,
                               w["projector_norm_beta"].ap(), KD, "b_pr")
        baggT = None
        if not triv["aggregator_bias"]:
            baggT = _dmajor(nc, singles, ps_tr, ident_bf,
                            w["aggregator_bias"].ap(), KD, "bagg")
        b1T = None
        if not triv["projector_dense1_bias"]:
            b1T = _dmajor(nc, singles, ps_tr, ident_bf,
                          w["projector_dense1_bias"].ap(), KH, "b1")
        b2T_rep = None
        if not triv["projector_dense2_bias"]:
            b2T = _dmajor(nc, singles, ps_tr, ident_bf,
                          w["projector_dense2_bias"].ap(), KD, "b2")
            b2T_rep = _rep0(b2T[:], NUM_PATHS, pos=2)  # [128, KD, P] view
        gbias_row = None
        if not triv["gate_bias"]:
            gbias_row = rows.tile([1, D], BF16, tag="brow")
            nc.gpsimd.dma_start(out=gbias_row[:],
                                in_=w["gate_bias"].ap().rearrange("(a d) -> a d", a=1))

        # ---------------- phase 1: load h (casting DMA) + context ----------------
        h_src = h_ext.ap().rearrange("(m t p) d -> p m t d", p=128, t=2)
        ctx_ps = ps_small.tile([128, KD], F32, tag="sm")
        for m2 in range(ML // 2):
            nc.sync.dma_start(out=h_bf[:, 2 * m2:2 * m2 + 2, :], in_=h_src[:, m2])
        for k in range(KD):
            for m in range(ML):
                nc.tensor.matmul(ctx_ps[:, k:k + 1],
                                 h_bf[:, m, k * 128:(k + 1) * 128],
                                 ones_bf[:],
                                 start=(m == 0), stop=(m == ML - 1))
        # ctxT[p, k] = context[k*128+p] = mean over L
        ctxT = singles.tile([128, KD], F32)
        nc.scalar.mul(ctxT[:], ctx_ps[:], 1.0 / L)
        if "d_ctxT" in dbg:
            nc.sync.dma_start(out=dbg["d_ctxT"].ap(), in_=ctxT[:])

        # ---------------- phase 2a: Wg (casting DMA) ----------------
        wg_src = w["gate_weight"].ap().rearrange("(k t p) d -> p k t d", p=128, t=2)
        for k2 in range(KD // 2):
            nc.sync.dma_start(out=wg_bf[:, 2 * k2:2 * k2 + 2, :], in_=wg_src[:, k2])

        # ---------------- phase 3: input LN + thought0 ----------------
        sqc = smalls.tile([128, KD], F32, tag="sqc")
        nc.vector.tensor_mul(sqc[:], ctxT[:], ctxT[:])
        cst_ps = ps_small.tile([1, 2 * KD], F32, tag="sm")
        nc.tensor.matmul(cst_ps[0:1, 0:KD], ones_f32[:], ctxT[:], start=True, stop=True)
        nc.tensor.matmul(cst_ps[0:1, KD:2 * KD], ones_f32[:], sqc[:], start=True, stop=True)
        csums = smalls.tile([1, 2], F32, tag="csums")
        nc.vector.tensor_reduce(csums[0:1, 0:1], cst_ps[0:1, 0:KD], axis=AX.X, op=ALU.add)
        nc.vector.tensor_reduce(csums[0:1, 1:2], cst_ps[0:1, KD:2 * KD], axis=AX.X, op=ALU.add)
        cmr = smalls.tile([1, 2], F32, tag="cmr")      # [mean, rstd]
        nc.scalar.mul(cmr[0:1, 0:1], csums[0:1, 0:1], 1.0 / D)
        csq = smalls.tile([1, 2], F32, tag="csq")
        nc.scalar.mul(csq[0:1, 0:1], csums[0:1, 1:2], 1.0 / D)   # E[x^2]
        nc.vector.tensor_mul(csq[0:1, 1:2], cmr[0:1, 0:1], cmr[0:1, 0:1])  # mean^2
        cvar = smalls.tile([1, 1], F32, tag="cvar")
        nc.vector.tensor_sub(cvar[:], csq[0:1, 0:1], csq[0:1, 1:2])
        nc.scalar.activation(cvar[:], cvar[:], AF.Sqrt, bias=eps1[0:1, :])
        nc.vector.reciprocal(cmr[0:1, 1:2], cvar[:])
        cmr_ps = ps_small.tile([128, 2], F32, tag="sm")
        nc.tensor.matmul(cmr_ps[:], ones_row[:], cmr[:], start=True, stop=True)
        cmr_b = smalls.tile([128, 2], F32, tag="cmrb")
        nc.scalar.copy(cmr_b[:], cmr_ps[:])
        ctxn = smalls.tile([128, KD], F32, tag="ctxn")
        nc.vector.tensor_scalar(ctxn[:], ctxT[:], cmr_b[:, 0:1], cmr_b[:, 1:2],
                                op0=ALU.subtract, op1=ALU.mult)
        if gammaT_in is not None:
            nc.vector.tensor_mul(ctxn[:], ctxn[:], gammaT_in[:])
            nc.vector.tensor_add(ctxn[:], ctxn[:], betaT_in[:])
        if "d_ctxn" in dbg:
            nc.sync.dma_start(out=dbg["d_ctxn"].ap(), in_=ctxn[:])
        ctxn_bf = smalls.tile([128, KD], BF16, tag="ctxnbf")
        nc.vector.tensor_copy(ctxn_bf[:], ctxn[:])

        # Wagg (casting DMA into the shared wab buffer)
        wagg_src = w["aggregator_weight"].ap().rearrange("(k t p) d -> p k t d",
                                                         p=128, t=2)
        for k2 in range(KD // 2):
            nc.sync.dma_start(out=wab_bf[:, 2 * k2:2 * k2 + 2, :],
                              in_=wagg_src[:, k2])
        # thought0 = ctx_n @ Wagg, d-major via per-k single-instr psum groups
        th0acc = smalls.tile([128, KD], F32, tag="th0acc")
        for k in range(KD):
            thp = ps_small.tile([128, KD], F32, tag="sm")
            for dm in range(KD):
                nc.tensor.matmul(thp[:, dm:dm + 1],
                                 wab_bf[:, k, dm * 128:(dm + 1) * 128],
                                 ctxn_bf[:, k:k + 1], start=True, stop=True)
            if k == 0:
                nc.vector.tensor_copy(th0acc[:], thp[:])
            else:
                nc.vector.tensor_add(th0acc[:], th0acc[:], thp[:])
        if baggT is not None:
            nc.vector.tensor_add(th0acc[:], th0acc[:], baggT[:])
        if "d_th0" in dbg:
            nc.sync.dma_start(out=dbg["d_th0"].ap(), in_=th0acc[:])
        # seed 8 paths: tT[p, k, q] = th0[k*128+p] * (1 + 0.02 q)
        tT = tstate.tile([128, KD, NUM_PATHS], F32, tag="tT")
        for q in range(NUM_PATHS):
            nc.scalar.mul(tT[:, :, q], th0acc[:], 1.0 + 0.02 * q)

        # ---------------- phase 7: gate matmul (spilled to DRAM) ----------------
        # hT via SBUF->SBUF DMA transpose; lhsT = hT chunks, rhs = Wg.
        for m in range(ML):
            hTm = hTm_pool.tile([128, KD, 128], BF16, tag="hTm")
            nc.scalar.dma_start_transpose(out=hTm[:], in_=h_bf[:, m, :])
            for n in range(2):
                g_ps = ps_gate.tile([128, 512], F32, tag="gps")
                for k in range(KD):
                    nc.tensor.matmul(g_ps[:], hTm[:, k, :],
                                     wg_bf[:, k, n * 512:(n + 1) * 512],
                                     start=(k == 0),
                                     stop=(k == KD - 1 and gbias_row is None))
                if gbias_row is not None:
                    nc.tensor.matmul(g_ps[:], ones_row_bf[:],
                                     gbias_row[0:1, n * 512:(n + 1) * 512],
                                     start=False, stop=True)
                g_sb = gout.tile([128, 512], BF16, tag="gout")
                nc.scalar.activation(g_sb[:], g_ps[:], AF.Sigmoid)
                nc.scalar.dma_start(
                    out=gate_dram[m * 128:(m + 1) * 128, n * 512:(n + 1) * 512],
                    in_=g_sb[:])

        # ---------------- phase 2c/2d: W1 / W2 (casting DMAs) ----------------
        w1_src = w["projector_dense1_weight"].ap().rearrange("(k p) h -> p k h", p=128)
        for k in range(KD):
            nc.sync.dma_start(out=w1_bf[:, k, :], in_=w1_src[:, k])
        w2_src = w["projector_dense2_weight"].ap().rearrange("(k t p) d -> p k t d",
                                                             p=128, t=4)
        for k4 in range(KH // 4):
            nc.sync.dma_start(out=w2_bf[:, 4 * k4:4 * k4 + 4, :], in_=w2_src[:, k4])

        # ---------------- phase 4: thought steps ----------------
        for step in range(NUM_STEPS):
            last = step == NUM_STEPS - 1
            sq = smalls.tile([128, KD, NUM_PATHS], F32, tag="sq")
            nc.vector.tensor_mul(sq[:], tT[:], tT[:])
            st_ps = ps_small.tile([1, 128], F32, tag="sm")
            nc.tensor.matmul(st_ps[0:1, 0:64], ones_f32[:],
                             tT[:].rearrange("a k q -> a q k"), start=True, stop=True)
            nc.tensor.matmul(st_ps[0:1, 64:128], ones_f32[:],
                             sq[:].rearrange("a k q -> a q k"), start=True, stop=True)
            sums = smalls.tile([1, 2 * NUM_PATHS], F32, tag="sums")
            nc.vector.tensor_reduce(sums[0:1, 0:NUM_PATHS],
                                    st_ps[0:1, 0:64].rearrange("a (q k) -> a q k", k=KD),
                                    axis=AX.X, op=ALU.add)
            nc.vector.tensor_reduce(sums[0:1, NUM_PATHS:],
                                    st_ps[0:1, 64:128].rearrange("a (q k) -> a q k", k=KD),
                                    axis=AX.X, op=ALU.add)
            mr = smalls.tile([1, 2 * NUM_PATHS], F32, tag="mr")  # [mean(8), rstd(8)]
            nc.scalar.mul(mr[0:1, 0:NUM_PATHS], sums[0:1, 0:NUM_PATHS], 1.0 / D)
            msq = smalls.tile([1, NUM_PATHS], F32, tag="msq")
            nc.scalar.mul(msq[0:1, :], sums[0:1, NUM_PATHS:], 1.0 / D)
            m2 = smalls.tile([1, NUM_PATHS], F32, tag="m2")
            nc.vector.tensor_mul(m2[0:1, :], mr[0:1, 0:NUM_PATHS], mr[0:1, 0:NUM_PATHS])
            var = smalls.tile([1, NUM_PATHS], F32, tag="var")
            nc.vector.tensor_sub(var[0:1, :], msq[0:1, :], m2[0:1, :])
            nc.scalar.activation(var[0:1, :], var[0:1, :], AF.Sqrt, bias=eps1[0:1, :])
            nc.vector.reciprocal(mr[0:1, NUM_PATHS:], var[0:1, :])
            mr_ps = ps_small.tile([128, 2 * NUM_PATHS], F32, tag="sm")
            nc.tensor.matmul(mr_ps[:], ones_row[:], mr[:], start=True, stop=True)
            mr_b = smalls.tile([128, 2 * NUM_PATHS], F32, tag="mrb")
            nc.scalar.copy(mr_b[:], mr_ps[:])

            # normalize all (k, q) at once with stride-0 broadcasts
            tn_bf = smalls.tile([128, KD, NUM_PATHS], BF16, tag="tnbf")
            tc_f = smalls.tile([128, KD, NUM_PATHS], F32, tag="tcf")
            nc.vector.tensor_tensor(out=tc_f[:], in0=tT[:],
                                    in1=_rep0(mr_b[:, 0:NUM_PATHS], KD),
                                    op=ALU.subtract)
            if gammaT_pr is not None:
                nc.vector.tensor_tensor(out=tc_f[:], in0=tc_f[:],
                                        in1=_rep0(mr_b[:, NUM_PATHS:], KD),
                                        op=ALU.mult)
                nc.vector.tensor_tensor(out=tc_f[:], in0=tc_f[:],
                                        in1=_rep0(gammaT_pr[:], NUM_PATHS, pos=2),
                                        op=ALU.mult)
                nc.vector.tensor_tensor(out=tn_bf[:], in0=tc_f[:],
                                        in1=_rep0(betaT_pr[:], NUM_PATHS, pos=2),
                                        op=ALU.add)
            else:
                nc.vector.tensor_tensor(out=tn_bf[:], in0=tc_f[:],
                                        in1=_rep0(mr_b[:, NUM_PATHS:], KD),
                                        op=ALU.mult)

            # dense1: x1 = gelu(tn @ W1 [+ b1]) in [H-part, path] layout
            x1_bf = smalls.tile([128, KH // 8, 8, NUM_PATHS], BF16, tag="x1")
            for tblk in range(KH // 8):
                x1_ps = ps_th.tile([128, 8 * NUM_PATHS], F32, tag="th")
                for hs in range(8):
                    mh = tblk * 8 + hs
                    for k in range(KD):
                        nc.tensor.matmul(x1_ps[:, hs * 8:(hs + 1) * 8],
                                         w1_bf[:, k, mh * 128:(mh + 1) * 128],
                                         tn_bf[:, k, :],
                                         start=(k == 0), stop=(k == KD - 1))
                # tanh-gelu (matches jax.nn.gelu approximate=True)
                xs = smalls.tile([128, 8 * NUM_PATHS], F32, tag="gelu_x")
                if b1T is not None:
                    for hs in range(8):
                        mh = tblk * 8 + hs
                        nc.scalar.activation(xs[:, hs * 8:(hs + 1) * 8],
                                             x1_ps[:, hs * 8:(hs + 1) * 8],
                                             AF.Identity, bias=b1T[:, mh:mh + 1])
                else:
                    nc.scalar.copy(xs[:], x1_ps[:])
                u = smalls.tile([128, 8 * NUM_PATHS], F32, tag="gelu_u")
                nc.vector.tensor_mul(u[:], xs[:], xs[:])
                nc.vector.tensor_mul(u[:], u[:], xs[:])
                nc.vector.scalar_tensor_tensor(u[:], u[:], 0.044715, xs[:],
                                               op0=ALU.mult, op1=ALU.add)
                nc.scalar.activation(u[:], u[:], AF.Tanh, scale=0.7978845608028654)
                nc.vector.scalar_tensor_tensor(u[:], u[:], 1.0, xs[:],
                                               op0=ALU.add, op1=ALU.mult)
                nc.scalar.mul(x1_bf[:, tblk].rearrange("a b c -> a (b c)"), u[:], 0.5)

            # dense2 + residual
            y_ps = ps_th.tile([128, KD * NUM_PATHS], F32, tag="th")
            for dm in range(KD):
                for hk in range(KH):
                    nc.tensor.matmul(y_ps[:, dm * 8:(dm + 1) * 8],
                                     w2_bf[:, hk, dm * 128:(dm + 1) * 128],
                                     x1_bf[:, hk // 8, hk % 8, :],
                                     start=(hk == 0), stop=(hk == KH - 1))
            tT_new = tstate.tile([128, KD, NUM_PATHS], F32, tag="tT")
            yv = y_ps[:].rearrange("a (k q) -> a k q", k=KD)
            if b2T_rep is not None:
                nc.vector.tensor_add(tT_new[:], yv, b2T_rep)
                nc.vector.tensor_add(tT_new[:], tT_new[:], tT[:])
            else:
                nc.vector.tensor_add(tT_new[:], yv, tT[:])
            tT = tT_new

            if last:
                sc_ps = ps_small.tile([1, NUM_PATHS], F32, tag="sm")
                for k in range(KD):
                    nc.tensor.matmul(sc_ps[:], ctxT[:, k:k + 1], tT[:, k, :],
                                     start=(k == 0), stop=(k == KD - 1))
                sc = smalls.tile([1, NUM_PATHS], F32, tag="sc")
                nc.scalar.mul(sc[:], sc_ps[:], INV_SQRT_D)
                negmax = smalls.tile([1, 1], F32, tag="negmax")
                nc.vector.tensor_reduce(negmax[:], sc[:], axis=AX.X, op=ALU.max,
                                        negate=True)
                esum = smalls.tile([1, 1], F32, tag="esum")
                ex = smalls.tile([1, NUM_PATHS], F32, tag="ex")
                nc.scalar.activation(ex[:], sc[:], AF.Exp, bias=negmax[0:1, :],
                                     accum_out=esum[:])
                rsum = smalls.tile([1, 1], F32, tag="rsum")
                nc.vector.reciprocal(rsum[:], esum[:])
                amps0 = smalls.tile([1, NUM_PATHS], F32, tag="amps0")
                nc.vector.tensor_scalar(amps0[:], ex[:], rsum[0:1, :], None, op0=ALU.mult)
                mask = smalls.tile([1, NUM_PATHS], F32, tag="mask")
                nc.vector.tensor_scalar(mask[:], amps0[:], PRUNE, None, op0=ALU.is_ge)
                pruned = smalls.tile([1, NUM_PATHS], F32, tag="pruned")
                nc.vector.tensor_mul(pruned[:], amps0[:], mask[:])
                psum_s = smalls.tile([1, 1], F32, tag="psums")
                nc.vector.tensor_reduce(psum_s[:], pruned[:], axis=AX.X, op=ALU.add)
                nc.vector.tensor_scalar(psum_s[:], psum_s[:], EPS, None, op0=ALU.add)
                rr = smalls.tile([1, 1], F32, tag="rr")
                nc.vector.reciprocal(rr[:], psum_s[:])
                ampsF = smalls.tile([1, NUM_PATHS], F32, tag="ampsF")
                nc.vector.tensor_scalar(ampsF[:], pruned[:], rr[0:1, :], None, op0=ALU.mult)

        if "d_tT" in dbg:
            nc.sync.dma_start(out=dbg["d_tT"].ap(),
                              in_=tT[:].rearrange("a k q -> a (k q)"))
        if "d_amps" in dbg:
            nc.sync.dma_start(out=dbg["d_amps"].ap(), in_=ampsF[:])

        # ---------------- phase 5: collapse + bc ----------------
        ab_ps = ps_small.tile([128, NUM_PATHS], F32, tag="sm")
        nc.tensor.matmul(ab_ps[:], ones_row[:], ampsF[:], start=True, stop=True)
        amps_sb = smalls.tile([128, NUM_PATHS], F32, tag="ampssb")
        nc.scalar.copy(amps_sb[:], ab_ps[:])
        prod = smalls.tile([128, KD, NUM_PATHS], F32, tag="prod")
        nc.vector.tensor_tensor(out=prod[:], in0=tT[:], in1=_rep0(amps_sb[:], KD),
                                op=ALU.mult)
        finalT = smalls.tile([128, KD], F32, tag="finalT")
        nc.vector.tensor_reduce(finalT[:], prod[:], axis=AX.X, op=ALU.add)
        finalT_bf = smalls.tile([128, KD], BF16, tag="finalTbf")
        nc.vector.tensor_copy(finalT_bf[:], finalT[:])

        # Wbc overwrites the shared wab buffer (casting DMA)
        wbc_src = w["broadcast_weight"].ap().rearrange("(k t p) d -> p k t d",
                                                       p=128, t=2)
        for k2 in range(KD // 2):
            nc.sync.dma_start(out=wab_bf[:, 2 * k2:2 * k2 + 2, :],
                              in_=wbc_src[:, k2])
        # bc row [1, D] in two 512-halves, then K=1 matmul broadcast
        bc_bf = singles.tile([128, D], BF16)
        for n in range(2):
            bc_ps = ps_gate.tile([1, 512], F32, tag="gps")
            for k in range(KD):
                nc.tensor.matmul(bc_ps[:],
                                 finalT_bf[:, k:k + 1],
                                 wab_bf[:, k, n * 512:(n + 1) * 512],
                                 start=(k == 0), stop=(k == KD - 1))
            bc_half = rows.tile([1, 512], F32, tag="row")
            if not triv["broadcast_bias"]:
                bbh = rows.tile([1, 512], F32, tag="brow")
                nc.sync.dma_start(
                    out=bbh[:],
                    in_=w["broadcast_bias"].ap()[n * 512:(n + 1) * 512]
                        .rearrange("(a d) -> a d", a=1))
                nc.vector.tensor_add(bc_half[:], bc_ps[:], bbh[:])
            else:
                nc.scalar.copy(bc_half[:], bc_ps[:])
            bcb_ps = ps_gate.tile([128, 512], F32, tag="gps")
            nc.tensor.matmul(bcb_ps[:], ones_row[:], bc_half[:], start=True, stop=True)
            nc.scalar.copy(bc_bf[:, n * 512:(n + 1) * 512], bcb_ps[:])
        if "d_bcbf" in dbg:
            nc.sync.dma_start(out=dbg["d_bcbf"].ap(), in_=bc_bf[:])

        # release the weight pool; final-phase pools reuse the space
        wpool.release()
        gin = ctx.enter_context(tc.tile_pool(name="gin", bufs=3))
        fin = ctx.enter_context(tc.tile_pool(name="fin", bufs=3))
        gamma_out_b = beta_out_b = None
        if not triv["output_norm"]:
            fin1 = ctx.enter_context(tc.tile_pool(name="fin1", bufs=1))
            gamma_out_b = fin1.tile([128, D], F32)
            nc.sync.dma_start(out=gamma_out_b[:], in_=_bc0(w["output_norm_gamma"].ap()))
            beta_out_b = fin1.tile([128, D], F32)
            nc.sync.dma_start(out=beta_out_b[:], in_=_bc0(w["output_norm_beta"].ap()))

        # ---------------- phase 8: final LN + output ----------------
        # all elementwise work in bf16 (DVE 4x mode); stats accumulate in f32
        for m in range(ML):
            g_in = gin.tile([128, D], BF16, tag="gin")
            nc.gpsimd.dma_start(out=g_in[:], in_=gate_dram[m * 128:(m + 1) * 128, :])
            if m == 0 and "d_gate0" in dbg:
                nc.sync.dma_start(out=dbg["d_gate0"].ap(), in_=g_in[:])
            p1 = gin.tile([128, D], BF16, tag="p1")
            nc.vector.tensor_mul(p1[:], g_in[:], bc_bf[:])
            # pre = p1 + h, with fused row-sum accumulation
            pre = fin.tile([128, D], BF16, tag="pre")
            rs = fin.tile([128, 2], F32, tag="rs")   # [rowsum, rowsumsq]
            nc.vector.scalar_tensor_tensor(pre[:], p1[:], 1.0, h_bf[:, m, :],
                                           op0=ALU.mult, op1=ALU.add,
                                           accum_out=rs[:, 0:1])
            if m == 0 and "d_pre0" in dbg:
                nc.sync.dma_start(out=dbg["d_pre0"].ap(), in_=pre[:])
            sqs = fin.tile([128, D], BF16, tag="sqs")
            nc.vector.scalar_tensor_tensor(sqs[:], pre[:], 1.0, pre[:],
                                           op0=ALU.mult, op1=ALU.mult,
                                           accum_out=rs[:, 1:2])
            mv = fin.tile([128, 2], F32, tag="mv")   # [mean, E[x^2]]
            nc.vector.tensor_scalar(mv[:], rs[:], 1.0 / D, None, op0=ALU.mult)
            var = fin.tile([128, 1], F32, tag="var")
            nc.vector.tensor_tensor(out=var[:], in0=mv[:, 0:1], in1=mv[:, 0:1],
                                    op=ALU.mult)
            nc.vector.tensor_sub(var[:], mv[:, 1:2], var[:])
            sd = fin.tile([128, 1], F32, tag="sd")
            nc.scalar.activation(sd[:], var[:], AF.Sqrt, bias=eps_col[:])
            rstd = fin.tile([128, 1], F32, tag="rstd")
            nc.vector.reciprocal(rstd[:], sd[:])
            nmr = fin.tile([128, 1], F32, tag="nmr")
            nc.vector.tensor_scalar(nmr[:], mv[:, 0:1], rstd[:, 0:1], -1.0,
                                    op0=ALU.mult, op1=ALU.mult)
            o = fin.tile([128, D], F32, tag="o")
            nc.scalar.activation(o[:], pre[:], AF.Identity,
                                 bias=nmr[:, 0:1], scale=rstd[:, 0:1])
            if gamma_out_b is not None:
                nc.vector.tensor_mul(o[:], o[:], gamma_out_b[:])
                nc.vector.tensor_add(o[:], o[:], beta_out_b[:])
            nc.sync.dma_start(out=out_ext.ap()[m * 128:(m + 1) * 128, :], in_=o[:])


def _triv_flags(inputs):
    def ones(x):
        return bool(np.all(np.asarray(x) == 1.0))

    def zeros(x):
        return bool(np.all(np.asarray(x) == 0.0))

    return {
        "input_norm": ones(inputs["input_norm_gamma"]) and zeros(inputs["input_norm_beta"]),
        "projector_norm": ones(inputs["projector_norm_gamma"]) and zeros(inputs["projector_norm_beta"]),
        "output_norm": ones(inputs["output_norm_gamma"]) and zeros(inputs["output_norm_beta"]),
        "aggregator_bias": zeros(inputs["aggregator_bias"]),
        "projector_dense1_bias": zeros(inputs["projector_dense1_bias"]),
        "projector_dense2_bias": zeros(inputs["projector_dense2_bias"]),
        "broadcast_bias": zeros(inputs["broadcast_bias"]),
        "gate_bias": zeros(inputs["gate_bias"]),
    }


_GRAPH_CACHE = {}

BF16_INPUTS = ("hidden_states", "aggregator_weight", "projector_dense1_weight",
               "projector_dense2_weight", "broadcast_weight", "gate_weight")


def prep_in_maps(inputs):
    """Build per-core input maps; big tensors are converted to bf16 on the
    host (round-to-nearest) so the NEFF reads half the bytes."""
    import ml_dtypes
    hs = np.ascontiguousarray(
        np.asarray(inputs["hidden_states"], dtype=np.float32).astype(ml_dtypes.bfloat16))
    assert hs.shape == (B, L, D)
    weights = {}
    for n in WEIGHT_NAMES:
        a = np.asarray(inputs[n], dtype=np.float32)
        if n in BF16_INPUTS:
            a = a.astype(ml_dtypes.bfloat16)
        weights[n] = np.ascontiguousarray(a)
    in_maps = []
    for b in range(B):
        m = {"hidden_states": np.ascontiguousarray(hs[b])}
        m.update(weights)
        in_maps.append(m)
    return in_maps


def kernel(**inputs):
    triv = _triv_flags(inputs)
    key = tuple(sorted(triv.items()))
    if key not in _GRAPH_CACHE:
        _GRAPH_CACHE[key] = build_graph(triv)
    nc = _GRAPH_CACHE[key]
    in_maps = prep_in_maps(inputs)
    res = run_bass_kernel_spmd(nc, in_maps, core_ids=list(range(B)))
    out = np.stack([res.results[b]["out"] for b in range(B)], axis=0)
    return out.astype(np.float32)


# revision 44
# speedup vs baseline: 1.1618x; 1.0503x over previous
"""Trainium2 Bass kernel for nn_ContinuousThoughtBlock.

Strategy: pure data-parallel over batch (B=8 -> 8 NeuronCores), zero
collectives.  Each core computes one batch element end-to-end:

  context = mean_L(h)                       [D]
  ctx_n   = LN(context); th0 = ctx_n @ Wagg [D]
  8 paths evolve through 4 residual-MLP steps (bf16 matmuls, weights
  stationary on the PE, activations in a [D-on-partitions, path] layout
  so LayerNorm stats become ones-vector matmuls / partition reductions)
  amps    = pruned softmax over paths (only needed after last step)
  bc      = (sum_p amps_p * th_p) @ Wbc     [D]
  gate    = sigmoid(h @ Wg + bg)            [L, D]  (bf16, spilled to DRAM)
  out     = LN_D(h + gate * bc)             [L, D]

Key mechanics:
  - all f32 weights/activations are loaded with gpsimd casting DMAs
    (f32 DRAM -> bf16 SBUF at line rate; no staging, no cast ops)
  - h is transposed for the gate matmul with SBUF->SBUF DMA-transpose
  - gate spills to DRAM (bf16) and is re-read in the final phase
  - three DMA queues: gpsimd SWDGE = weight/h loads + gate re-read,
    ACT HWDGE = hT transposes + gate spill, SP HWDGE = small vectors +
    output writes
  - LN statistics, softmax/prune, residual adds stay in f32
"""

import numpy as np

import concourse.bass as bass
import concourse.mybir as mybir
import concourse.tile as tile
from concourse import bacc
from concourse.bass_utils import run_bass_kernel_spmd
from concourse.masks import make_identity

# Problem constants (hardcoded per harness contract).
B, L, D, H = 8, 2048, 1024, 4096
NUM_PATHS = 8
NUM_STEPS = 4
PRUNE = 0.1
EPS = 1e-6
KD = D // 128    # 8  D-chunks
KH = H // 128    # 32 H-chunks
ML = L // 128    # 16 L-tiles
INV_SQRT_D = 1.0 / float(np.sqrt(np.float32(D)))

F32 = mybir.dt.float32
BF16 = mybir.dt.bfloat16
AF = mybir.ActivationFunctionType
ALU = mybir.AluOpType
AX = mybir.AxisListType

WEIGHT_NAMES = [
    "input_norm_gamma", "input_norm_beta",
    "aggregator_weight", "aggregator_bias",
    "projector_norm_gamma", "projector_norm_beta",
    "projector_dense1_weight", "projector_dense1_bias",
    "projector_dense2_weight", "projector_dense2_bias",
    "broadcast_weight", "broadcast_bias",
    "gate_weight", "gate_bias",
    "output_norm_gamma", "output_norm_beta",
]


def _bc0(ap, n=128):
    """Broadcast a 1-D AP down n partitions via a stride-0 partition dim."""
    return bass.AP(tensor=ap.tensor, offset=ap.offset, ap=[[0, n]] + list(ap.ap))


def _rep0(ap, n, pos=1):
    """Insert a stride-0 free dim of extent n at position pos."""
    new = list(ap.ap)
    new.insert(pos, [0, n])
    return bass.AP(tensor=ap.tensor, offset=ap.offset, ap=new)


def build_graph(triv, debug=False):
    nc = bacc.Bacc("TRN2", target_bir_lowering=False, debug=False,
                   enable_asserts=True, num_devices=B)

    h_ext = nc.declare_dram_parameter("hidden_states", [L, D], BF16, isOutput=False)
    w_ext = {}
    w_ext["aggregator_weight"] = nc.declare_dram_parameter(
        "aggregator_weight", [D, D], BF16, isOutput=False)
    w_ext["projector_dense1_weight"] = nc.declare_dram_parameter(
        "projector_dense1_weight", [D, H], BF16, isOutput=False)
    w_ext["projector_dense2_weight"] = nc.declare_dram_parameter(
        "projector_dense2_weight", [H, D], BF16, isOutput=False)
    w_ext["broadcast_weight"] = nc.declare_dram_parameter(
        "broadcast_weight", [D, D], BF16, isOutput=False)
    w_ext["gate_weight"] = nc.declare_dram_parameter(
        "gate_weight", [D, D], BF16, isOutput=False)
    for n in ("input_norm_gamma", "input_norm_beta", "aggregator_bias",
              "projector_norm_gamma", "projector_norm_beta",
              "projector_dense1_bias", "projector_dense2_bias",
              "broadcast_bias", "gate_bias",
              "output_norm_gamma", "output_norm_beta"):
        shape = [H] if n == "projector_dense1_bias" else [D]
        w_ext[n] = nc.declare_dram_parameter(n, shape, F32, isOutput=False)
    out_ext = nc.declare_dram_parameter("out", [L, D], F32, isOutput=True)
    dbg = {}
    if debug:
        for nm, shape in (("d_ctxT", [128, KD]), ("d_ctxn", [128, KD]),
                          ("d_th0", [128, KD]), ("d_tT", [128, KD * NUM_PATHS]),
                          ("d_amps", [1, NUM_PATHS]), ("d_bcbf", [128, D]),
                          ("d_gate0", [128, D]), ("d_pre0", [128, D])):
            dbg[nm] = nc.declare_dram_parameter(nm, shape, F32, isOutput=True)

    with tile.TileContext(nc) as tc:
        _build_body(nc, tc, h_ext, w_ext, out_ext, triv, dbg)
    nc.compile()
    return nc


def _dmajor(nc, pool, ps_pool, ident_bf, dram_ap, n, name):
    """DMA a [n*128] DRAM vector into a [128, n] d-major SBUF tile
    (tile[p, k] = v[k*128 + p]) via a bf16 [n,128] load + PE transpose.
    Values are bf16-rounded, acceptable for gamma/beta/bias vectors."""
    rowk = pool.tile([n, 128], BF16, tag="dmaj_rowk")
    nc.gpsimd.dma_start(out=rowk[:], in_=dram_ap.rearrange("(k p) -> k p", p=128))
    ps = ps_pool.tile([128, n], BF16, tag="tr")
    nc.tensor.transpose(ps[:], rowk[:], ident_bf[0:n, 0:n])
    t = pool.tile([128, n], F32, tag=name)
    nc.scalar.copy(t[:], ps[:])
    return t


def _build_body(nc, tc, h_ext, w, out_ext, triv, dbg=None):
    dbg = dbg or {}
    import contextlib
    ctx = contextlib.ExitStack()
    with ctx:
        # ---------------- pools ----------------
        singles = ctx.enter_context(tc.tile_pool(name="singles", bufs=1))
        smalls = ctx.enter_context(tc.tile_pool(name="smalls", bufs=1))
        tstate = ctx.enter_context(tc.tile_pool(name="tstate", bufs=2))
        all_triv = all(triv.values())
        hTm_pool = ctx.enter_context(tc.tile_pool(name="hTm", bufs=2 if all_triv else 1))
        gout = ctx.enter_context(tc.tile_pool(name="gout", bufs=2 if all_triv else 1))
        rows = ctx.enter_context(tc.tile_pool(name="rows", bufs=1))
        wpool = tc.alloc_tile_pool(name="wpool", bufs=1)
        dram = ctx.enter_context(tc.tile_pool(name="dram", bufs=1, space="DRAM"))

        ps_small = ctx.enter_context(tc.tile_pool(name="ps_small", bufs=1, space="PSUM"))
        ps_tr = ctx.enter_context(tc.tile_pool(name="ps_tr", bufs=1, space="PSUM"))
        ps_gate = ctx.enter_context(tc.tile_pool(name="ps_gate", bufs=4, space="PSUM"))
        ps_th = ctx.enter_context(tc.tile_pool(name="ps_th", bufs=2, space="PSUM"))

        # ---------------- constants ----------------
        ident_bf = singles.tile([128, 128], BF16)
        make_identity(nc, ident_bf[:])
        ones_bf = singles.tile([128, 1], BF16)
        nc.vector.memset(ones_bf[:], 1.0)
        ones_f32 = singles.tile([128, 1], F32)
        nc.vector.memset(ones_f32[:], 1.0)
        ones_row = singles.tile([1, 128], F32)
        nc.vector.memset(ones_row[:], 1.0)
        ones_row_bf = singles.tile([1, 128], BF16)
        nc.vector.memset(ones_row_bf[:], 1.0)
        eps1 = singles.tile([1, 1], F32)
        nc.vector.memset(eps1[:], EPS)
        eps_col = singles.tile([128, 1], F32)
        nc.vector.memset(eps_col[:], EPS)

        # resident (bf16) tensors
        h_bf = singles.tile([128, ML, D], BF16)      # 32KB/part
        w1_bf = wpool.tile([128, KD, H], BF16)       # 64KB/part
        w2_bf = wpool.tile([128, KH, D], BF16)       # 64KB/part
        wg_bf = wpool.tile([128, KD, D], BF16)       # 16KB/part
        wab_bf = wpool.tile([128, KD, D], BF16)      # 16KB/part (Wagg, later Wbc)

        gate_dram = dram.tile([L, D], BF16)

        # d-major vectors (only when nontrivial)
        gammaT_in = betaT_in = None
        if not triv["input_norm"]:
            gammaT_in = _dmajor(nc, singles, ps_tr, ident_bf,
                                w["input_norm_gamma"].ap(), KD, "g_in")
            betaT_in = _dmajor(nc, singles, ps_tr, ident_bf,
                               w["input_norm_beta"].ap(), KD, "b_in")
        gammaT_pr = betaT_pr = None
        if not triv["projector_norm"]:
            gammaT_pr = _dmajor(nc, singles, ps_tr, ident_bf,
                                w["projector_norm_gamma"].ap(), KD, "g_pr")
            betaT_pr = _dmajor(nc, singles, ps_tr, ident_bf,
                               w["projector_norm_beta"].ap(), KD, "b_pr")
        baggT = None
        if not triv["aggregator_bias"]:
            baggT = _dmajor(nc, singles, ps_tr, ident_bf,
                            w["aggregator_bias"].ap(), KD, "bagg")
        b1T = None
        if not triv["projector_dense1_bias"]:
            b1T = _dmajor(nc, singles, ps_tr, ident_bf,
                          w["projector_dense1_bias"].ap(), KH, "b1")
        b2T_rep = None
        if not triv["projector_dense2_bias"]:
            b2T = _dmajor(nc, singles, ps_tr, ident_bf,
                          w["projector_dense2_bias"].ap(), KD, "b2")
            b2T_rep = _rep0(b2T[:], NUM_PATHS, pos=2)  # [128, KD, P] view
        gbias_row = None
        if not triv["gate_bias"]:
            gbias_row = rows.tile([1, D], BF16, tag="brow")
            nc.gpsimd.dma_start(out=gbias_row[:],
                                in_=w["gate_bias"].ap().rearrange("(a d) -> a d", a=1))

        # ---------------- phase 1: load h (casting DMA) + context ----------------
        h_src = h_ext.ap().rearrange("(m t p) d -> p m t d", p=128, t=2)
        ctx_ps = ps_small.tile([128, KD], F32, tag="sm")
        for m2 in range(ML // 2):
            nc.sync.dma_start(out=h_bf[:, 2 * m2:2 * m2 + 2, :], in_=h_src[:, m2])
        for k in range(KD):
            for m in range(ML):
                nc.tensor.matmul(ctx_ps[:, k:k + 1],
                                 h_bf[:, m, k * 128:(k + 1) * 128],
                                 ones_bf[:],
                                 start=(m == 0), stop=(m == ML - 1))
        # ctxT[p, k] = context[k*128+p] = mean over L
        ctxT = singles.tile([128, KD], F32)
        nc.scalar.mul(ctxT[:], ctx_ps[:], 1.0 / L)
        if "d_ctxT" in dbg:
            nc.sync.dma_start(out=dbg["d_ctxT"].ap(), in_=ctxT[:])

        # ---------------- phase 2a: Wg (casting DMA) ----------------
        wg_src = w["gate_weight"].ap().rearrange("(k t p) d -> p k t d", p=128, t=2)
        for k2 in range(KD // 2):
            nc.gpsimd.dma_start(out=wg_bf[:, 2 * k2:2 * k2 + 2, :], in_=wg_src[:, k2])

        # ---------------- phase 3: input LN + thought0 ----------------
        sqc = smalls.tile([128, KD], F32, tag="sqc")
        nc.vector.tensor_mul(sqc[:], ctxT[:], ctxT[:])
        cst_ps = ps_small.tile([1, 2 * KD], F32, tag="sm")
        nc.tensor.matmul(cst_ps[0:1, 0:KD], ones_f32[:], ctxT[:], start=True, stop=True)
        nc.tensor.matmul(cst_ps[0:1, KD:2 * KD], ones_f32[:], sqc[:], start=True, stop=True)
        csums = smalls.tile([1, 2], F32, tag="csums")
        nc.vector.tensor_reduce(csums[0:1, 0:1], cst_ps[0:1, 0:KD], axis=AX.X, op=ALU.add)
        nc.vector.tensor_reduce(csums[0:1, 1:2], cst_ps[0:1, KD:2 * KD], axis=AX.X, op=ALU.add)
        cmr = smalls.tile([1, 2], F32, tag="cmr")      # [mean, rstd]
        nc.scalar.mul(cmr[0:1, 0:1], csums[0:1, 0:1], 1.0 / D)
        csq = smalls.tile([1, 2], F32, tag="csq")
        nc.scalar.mul(csq[0:1, 0:1], csums[0:1, 1:2], 1.0 / D)   # E[x^2]
        nc.vector.tensor_mul(csq[0:1, 1:2], cmr[0:1, 0:1], cmr[0:1, 0:1])  # mean^2
        cvar = smalls.tile([1, 1], F32, tag="cvar")
        nc.vector.tensor_sub(cvar[:], csq[0:1, 0:1], csq[0:1, 1:2])
        nc.scalar.activation(cvar[:], cvar[:], AF.Sqrt, bias=eps1[0:1, :])
        nc.vector.reciprocal(cmr[0:1, 1:2], cvar[:])
        cmr_ps = ps_small.tile([128, 2], F32, tag="sm")
        nc.tensor.matmul(cmr_ps[:], ones_row[:], cmr[:], start=True, stop=True)
        cmr_b = smalls.tile([128, 2], F32, tag="cmrb")
        nc.scalar.copy(cmr_b[:], cmr_ps[:])
        ctxn = smalls.tile([128, KD], F32, tag="ctxn")
        nc.vector.tensor_scalar(ctxn[:], ctxT[:], cmr_b[:, 0:1], cmr_b[:, 1:2],
                                op0=ALU.subtract, op1=ALU.mult)
        if gammaT_in is not None:
            nc.vector.tensor_mul(ctxn[:], ctxn[:], gammaT_in[:])
            nc.vector.tensor_add(ctxn[:], ctxn[:], betaT_in[:])
        if "d_ctxn" in dbg:
            nc.sync.dma_start(out=dbg["d_ctxn"].ap(), in_=ctxn[:])
        ctxn_bf = smalls.tile([128, KD], BF16, tag="ctxnbf")
        nc.vector.tensor_copy(ctxn_bf[:], ctxn[:])

        # Wagg (casting DMA into the shared wab buffer)
        wagg_src = w["aggregator_weight"].ap().rearrange("(k t p) d -> p k t d",
                                                         p=128, t=2)
        for k2 in range(KD // 2):
            nc.gpsimd.dma_start(out=wab_bf[:, 2 * k2:2 * k2 + 2, :],
                                in_=wagg_src[:, k2])
        # thought0 = ctx_n @ Wagg, d-major via per-k single-instr psum groups
        th0acc = smalls.tile([128, KD], F32, tag="th0acc")
        for k in range(KD):
            thp = ps_small.tile([128, KD], F32, tag="sm")
            for dm in range(KD):
                nc.tensor.matmul(thp[:, dm:dm + 1],
                                 wab_bf[:, k, dm * 128:(dm + 1) * 128],
                                 ctxn_bf[:, k:k + 1], start=True, stop=True)
            if k == 0:
                nc.vector.tensor_copy(th0acc[:], thp[:])
            else:
                nc.vector.tensor_add(th0acc[:], th0acc[:], thp[:])
        if baggT is not None:
            nc.vector.tensor_add(th0acc[:], th0acc[:], baggT[:])
        if "d_th0" in dbg:
            nc.sync.dma_start(out=dbg["d_th0"].ap(), in_=th0acc[:])
        # seed 8 paths: tT[p, k, q] = th0[k*128+p] * (1 + 0.02 q)
        tT = tstate.tile([128, KD, NUM_PATHS], F32, tag="tT")
        for q in range(NUM_PATHS):
            nc.scalar.mul(tT[:, :, q], th0acc[:], 1.0 + 0.02 * q)

        # ---------------- phase 7: gate matmul (spilled to DRAM) ----------------
        # hT via SBUF->SBUF DMA transpose; lhsT = hT chunks, rhs = Wg.
        for m in range(ML):
            hTm = hTm_pool.tile([128, KD, 128], BF16, tag="hTm")
            nc.scalar.dma_start_transpose(out=hTm[:], in_=h_bf[:, m, :])
            for n in range(2):
                g_ps = ps_gate.tile([128, 512], F32, tag="gps")
                for k in range(KD):
                    nc.tensor.matmul(g_ps[:], hTm[:, k, :],
                                     wg_bf[:, k, n * 512:(n + 1) * 512],
                                     start=(k == 0),
                                     stop=(k == KD - 1 and gbias_row is None))
                if gbias_row is not None:
                    nc.tensor.matmul(g_ps[:], ones_row_bf[:],
                                     gbias_row[0:1, n * 512:(n + 1) * 512],
                                     start=False, stop=True)
                g_sb = gout.tile([128, 512], BF16, tag="gout")
                nc.scalar.activation(g_sb[:], g_ps[:], AF.Sigmoid)
                nc.scalar.dma_start(
                    out=gate_dram[m * 128:(m + 1) * 128, n * 512:(n + 1) * 512],
                    in_=g_sb[:])

        # ---------------- phase 2c/2d: W1 / W2 (casting DMAs) ----------------
        w1_src = w["projector_dense1_weight"].ap().rearrange("(k p) h -> p k h", p=128)
        for k in range(KD):
            nc.sync.dma_start(out=w1_bf[:, k, :], in_=w1_src[:, k])
        w2_src = w["projector_dense2_weight"].ap().rearrange("(k t p) d -> p k t d",
                                                             p=128, t=4)
        for k4 in range(KH // 4):
            nc.gpsimd.dma_start(out=w2_bf[:, 4 * k4:4 * k4 + 4, :], in_=w2_src[:, k4])

        # ---------------- phase 4: thought steps ----------------
        for step in range(NUM_STEPS):
            last = step == NUM_STEPS - 1
            sq = smalls.tile([128, KD, NUM_PATHS], F32, tag="sq")
            nc.vector.tensor_mul(sq[:], tT[:], tT[:])
            st_ps = ps_small.tile([1, 128], F32, tag="sm")
            nc.tensor.matmul(st_ps[0:1, 0:64], ones_f32[:],
                             tT[:].rearrange("a k q -> a q k"), start=True, stop=True)
            nc.tensor.matmul(st_ps[0:1, 64:128], ones_f32[:],
                             sq[:].rearrange("a k q -> a q k"), start=True, stop=True)
            sums = smalls.tile([1, 2 * NUM_PATHS], F32, tag="sums")
            nc.vector.tensor_reduce(sums[0:1, 0:NUM_PATHS],
                                    st_ps[0:1, 0:64].rearrange("a (q k) -> a q k", k=KD),
                                    axis=AX.X, op=ALU.add)
            nc.vector.tensor_reduce(sums[0:1, NUM_PATHS:],
                                    st_ps[0:1, 64:128].rearrange("a (q k) -> a q k", k=KD),
                                    axis=AX.X, op=ALU.add)
            mr = smalls.tile([1, 2 * NUM_PATHS], F32, tag="mr")  # [mean(8), rstd(8)]
            nc.scalar.mul(mr[0:1, 0:NUM_PATHS], sums[0:1, 0:NUM_PATHS], 1.0 / D)
            msq = smalls.tile([1, NUM_PATHS], F32, tag="msq")
            nc.scalar.mul(msq[0:1, :], sums[0:1, NUM_PATHS:], 1.0 / D)
            m2 = smalls.tile([1, NUM_PATHS], F32, tag="m2")
            nc.vector.tensor_mul(m2[0:1, :], mr[0:1, 0:NUM_PATHS], mr[0:1, 0:NUM_PATHS])
            var = smalls.tile([1, NUM_PATHS], F32, tag="var")
            nc.vector.tensor_sub(var[0:1, :], msq[0:1, :], m2[0:1, :])
            nc.scalar.activation(var[0:1, :], var[0:1, :], AF.Sqrt, bias=eps1[0:1, :])
            nc.vector.reciprocal(mr[0:1, NUM_PATHS:], var[0:1, :])
            mr_ps = ps_small.tile([128, 2 * NUM_PATHS], F32, tag="sm")
            nc.tensor.matmul(mr_ps[:], ones_row[:], mr[:], start=True, stop=True)
            mr_b = smalls.tile([128, 2 * NUM_PATHS], F32, tag="mrb")
            nc.scalar.copy(mr_b[:], mr_ps[:])

            # normalize all (k, q) at once with stride-0 broadcasts
            tn_bf = smalls.tile([128, KD, NUM_PATHS], BF16, tag="tnbf")
            tc_f = smalls.tile([128, KD, NUM_PATHS], F32, tag="tcf")
            nc.vector.tensor_tensor(out=tc_f[:], in0=tT[:],
                                    in1=_rep0(mr_b[:, 0:NUM_PATHS], KD),
                                    op=ALU.subtract)
            if gammaT_pr is not None:
                nc.vector.tensor_tensor(out=tc_f[:], in0=tc_f[:],
                                        in1=_rep0(mr_b[:, NUM_PATHS:], KD),
                                        op=ALU.mult)
                nc.vector.tensor_tensor(out=tc_f[:], in0=tc_f[:],
                                        in1=_rep0(gammaT_pr[:], NUM_PATHS, pos=2),
                                        op=ALU.mult)
                nc.vector.tensor_tensor(out=tn_bf[:], in0=tc_f[:],
                                        in1=_rep0(betaT_pr[:], NUM_PATHS, pos=2),
                                        op=ALU.add)
            else:
                nc.vector.tensor_tensor(out=tn_bf[:], in0=tc_f[:],
                                        in1=_rep0(mr_b[:, NUM_PATHS:], KD),
                                        op=ALU.mult)

            # dense1: x1 = gelu(tn @ W1 [+ b1]) in [H-part, path] layout
            x1_bf = smalls.tile([128, KH // 8, 8, NUM_PATHS], BF16, tag="x1")
            for tblk in range(KH // 8):
                x1_ps = ps_th.tile([128, 8 * NUM_PATHS], F32, tag="th")
                for hs in range(8):
                    mh = tblk * 8 + hs
                    for k in range(KD):
                        nc.tensor.matmul(x1_ps[:, hs * 8:(hs + 1) * 8],
                                         w1_bf[:, k, mh * 128:(mh + 1) * 128],
                                         tn_bf[:, k, :],
                                         start=(k == 0), stop=(k == KD - 1))
                # tanh-gelu (matches jax.nn.gelu approximate=True)
                xs = smalls.tile([128, 8 * NUM_PATHS], F32, tag="gelu_x")
                if b1T is not None:
                    for hs in range(8):
                        mh = tblk * 8 + hs
                        nc.scalar.activation(xs[:, hs * 8:(hs + 1) * 8],
                                             x1_ps[:, hs * 8:(hs + 1) * 8],
                                             AF.Identity, bias=b1T[:, mh:mh + 1])
                else:
                    nc.scalar.copy(xs[:], x1_ps[:])
                u = smalls.tile([128, 8 * NUM_PATHS], F32, tag="gelu_u")
                nc.vector.tensor_mul(u[:], xs[:], xs[:])
                nc.vector.tensor_mul(u[:], u[:], xs[:])
                nc.vector.scalar_tensor_tensor(u[:], u[:], 0.044715, xs[:],
                                               op0=ALU.mult, op1=ALU.add)
                nc.scalar.activation(u[:], u[:], AF.Tanh, scale=0.7978845608028654)
                nc.vector.scalar_tensor_tensor(u[:], u[:], 1.0, xs[:],
                                               op0=ALU.add, op1=ALU.mult)
                nc.scalar.mul(x1_bf[:, tblk].rearrange("a b c -> a (b c)"), u[:], 0.5)

            # dense2 + residual
            y_ps = ps_th.tile([128, KD * NUM_PATHS], F32, tag="th")
            for dm in range(KD):
                for hk in range(KH):
                    nc.tensor.matmul(y_ps[:, dm * 8:(dm + 1) * 8],
                                     w2_bf[:, hk, dm * 128:(dm + 1) * 128],
                                     x1_bf[:, hk // 8, hk % 8, :],
                                     start=(hk == 0), stop=(hk == KH - 1))
            tT_new = tstate.tile([128, KD, NUM_PATHS], F32, tag="tT")
            yv = y_ps[:].rearrange("a (k q) -> a k q", k=KD)
            if b2T_rep is not None:
                nc.vector.tensor_add(tT_new[:], yv, b2T_rep)
                nc.vector.tensor_add(tT_new[:], tT_new[:], tT[:])
            else:
                nc.vector.tensor_add(tT_new[:], yv, tT[:])
            tT = tT_new

            if last:
                sc_ps = ps_small.tile([1, NUM_PATHS], F32, tag="sm")
                for k in range(KD):
                    nc.tensor.matmul(sc_ps[:], ctxT[:, k:k + 1], tT[:, k, :],
                                     start=(k == 0), stop=(k == KD - 1))
                sc = smalls.tile([1, NUM_PATHS], F32, tag="sc")
                nc.scalar.mul(sc[:], sc_ps[:], INV_SQRT_D)
                negmax = smalls.tile([1, 1], F32, tag="negmax")
                nc.vector.tensor_reduce(negmax[:], sc[:], axis=AX.X, op=ALU.max,
                                        negate=True)
                esum = smalls.tile([1, 1], F32, tag="esum")
                ex = smalls.tile([1, NUM_PATHS], F32, tag="ex")
                nc.scalar.activation(ex[:], sc[:], AF.Exp, bias=negmax[0:1, :],
                                     accum_out=esum[:])
                rsum = smalls.tile([1, 1], F32, tag="rsum")
                nc.vector.reciprocal(rsum[:], esum[:])
                amps0 = smalls.tile([1, NUM_PATHS], F32, tag="amps0")
                nc.vector.tensor_scalar(amps0[:], ex[:], rsum[0:1, :], None, op0=ALU.mult)
                mask = smalls.tile([1, NUM_PATHS], F32, tag="mask")
                nc.vector.tensor_scalar(mask[:], amps0[:], PRUNE, None, op0=ALU.is_ge)
                pruned = smalls.tile([1, NUM_PATHS], F32, tag="pruned")
                nc.vector.tensor_mul(pruned[:], amps0[:], mask[:])
                psum_s = smalls.tile([1, 1], F32, tag="psums")
                nc.vector.tensor_reduce(psum_s[:], pruned[:], axis=AX.X, op=ALU.add)
                nc.vector.tensor_scalar(psum_s[:], psum_s[:], EPS, None, op0=ALU.add)
                rr = smalls.tile([1, 1], F32, tag="rr")
                nc.vector.reciprocal(rr[:], psum_s[:])
                ampsF = smalls.tile([1, NUM_PATHS], F32, tag="ampsF")
                nc.vector.tensor_scalar(ampsF[:], pruned[:], rr[0:1, :], None, op0=ALU.mult)

        if "d_tT" in dbg:
            nc.sync.dma_start(out=dbg["d_tT"].ap(),
                              in_=tT[:].rearrange("a k q -> a (k q)"))
        if "d_amps" in dbg:
            nc.sync.dma_start(out=dbg["d_amps"].ap(), in_=ampsF[:])

        # ---------------- phase 5: collapse + bc ----------------
        ab_ps = ps_small.tile([128, NUM_PATHS], F32, tag="sm")
        nc.tensor.matmul(ab_ps[:], ones_row[:], ampsF[:], start=True, stop=True)
        amps_sb = smalls.tile([128, NUM_PATHS], F32, tag="ampssb")
        nc.scalar.copy(amps_sb[:], ab_ps[:])
        prod = smalls.tile([128, KD, NUM_PATHS], F32, tag="prod")
        nc.vector.tensor_tensor(out=prod[:], in0=tT[:], in1=_rep0(amps_sb[:], KD),
                                op=ALU.mult)
        finalT = smalls.tile([128, KD], F32, tag="finalT")
        nc.vector.tensor_reduce(finalT[:], prod[:], axis=AX.X, op=ALU.add)
        finalT_bf = smalls.tile([128, KD], BF16, tag="finalTbf")
        nc.vector.tensor_copy(finalT_bf[:], finalT[:])

        # Wbc overwrites the shared wab buffer (casting DMA)
        wbc_src = w["broadcast_weight"].ap().rearrange("(k t p) d -> p k t d",
                                                       p=128, t=2)
        for k2 in range(KD // 2):
            nc.gpsimd.dma_start(out=wab_bf[:, 2 * k2:2 * k2 + 2, :],
                                in_=wbc_src[:, k2])
        # bc row [1, D] in two 512-halves, then K=1 matmul broadcast
        bc_bf = singles.tile([128, D], BF16)
        for n in range(2):
            bc_ps = ps_gate.tile([1, 512], F32, tag="gps")
            for k in range(KD):
                nc.tensor.matmul(bc_ps[:],
                                 finalT_bf[:, k:k + 1],
                                 wab_bf[:, k, n * 512:(n + 1) * 512],
                                 start=(k == 0), stop=(k == KD - 1))
            bc_half = rows.tile([1, 512], F32, tag="row")
            if not triv["broadcast_bias"]:
                bbh = rows.tile([1, 512], F32, tag="brow")
                nc.sync.dma_start(
                    out=bbh[:],
                    in_=w["broadcast_bias"].ap()[n * 512:(n + 1) * 512]
                        .rearrange("(a d) -> a d", a=1))
                nc.vector.tensor_add(bc_half[:], bc_ps[:], bbh[:])
            else:
                nc.scalar.copy(bc_half[:], bc_ps[:])
            bcb_ps = ps_gate.tile([128, 512], F32, tag="gps")
            nc.tensor.matmul(bcb_ps[:], ones_row[:], bc_half[:], start=True, stop=True)
            nc.scalar.copy(bc_bf[:, n * 512:(n + 1) * 512], bcb_ps[:])
        if "d_bcbf" in dbg:
            nc.sync.dma_start(out=dbg["d_bcbf"].ap(), in_=bc_bf[:])

        # release the weight pool; final-phase pools reuse the space
        wpool.release()
        gin = ctx.enter_context(tc.tile_pool(name="gin", bufs=3))
        fin = ctx.enter_context(tc.tile_pool(name="fin", bufs=3))
        gamma_out_b = beta_out_b = None
        if not triv["output_norm"]:
            fin1 = ctx.enter_context(tc.tile_pool(name="fin1", bufs=1))
            gamma_out_b = fin1.tile([128, D], F32)
            nc.sync.dma_start(out=gamma_out_b[:], in_=_bc0(w["output_norm_gamma"].ap()))
            beta_out_b = fin1.tile([128, D], F32)
            nc.sync.dma_start(out=beta_out_b[:], in_=_bc0(w["output_norm_beta"].ap()))

        # ---------------- phase 8: final LN + output ----------------
        # all elementwise work in bf16 (DVE 4x mode); stats accumulate in f32
        for m in range(ML):
            g_in = gin.tile([128, D], BF16, tag="gin")
            nc.gpsimd.dma_start(out=g_in[:], in_=gate_dram[m * 128:(m + 1) * 128, :])
            if m == 0 and "d_gate0" in dbg:
                nc.sync.dma_start(out=dbg["d_gate0"].ap(), in_=g_in[:])
            p1 = gin.tile([128, D], BF16, tag="p1")
            nc.vector.tensor_mul(p1[:], g_in[:], bc_bf[:])
            # pre = p1 + h, with fused row-sum accumulation
            pre = fin.tile([128, D], BF16, tag="pre")
            rs = fin.tile([128, 2], F32, tag="rs")   # [rowsum, rowsumsq]
            nc.vector.scalar_tensor_tensor(pre[:], p1[:], 1.0, h_bf[:, m, :],
                                           op0=ALU.mult, op1=ALU.add,
                                           accum_out=rs[:, 0:1])
            if m == 0 and "d_pre0" in dbg:
                nc.sync.dma_start(out=dbg["d_pre0"].ap(), in_=pre[:])
            sqs = fin.tile([128, D], BF16, tag="sqs")
            nc.scalar.activation(sqs[:], pre[:], AF.Square, accum_out=rs[:, 1:2])
            mv = fin.tile([128, 2], F32, tag="mv")   # [mean, E[x^2]]
            nc.vector.tensor_scalar(mv[:], rs[:], 1.0 / D, None, op0=ALU.mult)
            var = fin.tile([128, 1], F32, tag="var")
            nc.vector.tensor_tensor(out=var[:], in0=mv[:, 0:1], in1=mv[:, 0:1],
                                    op=ALU.mult)
            nc.vector.tensor_sub(var[:], mv[:, 1:2], var[:])
            sd = fin.tile([128, 1], F32, tag="sd")
            nc.scalar.activation(sd[:], var[:], AF.Sqrt, bias=eps_col[:])
            rstd = fin.tile([128, 1], F32, tag="rstd")
            nc.vector.reciprocal(rstd[:], sd[:])
            o = fin.tile([128, D], F32, tag="o")
            nc.vector.tensor_scalar(o[:], pre[:], mv[:, 0:1], rstd[:, 0:1],
                                    op0=ALU.subtract, op1=ALU.mult)
            if gamma_out_b is not None:
                nc.vector.tensor_mul(o[:], o[:], gamma_out_b[:])
                nc.vector.tensor_add(o[:], o[:], beta_out_b[:])
            nc.sync.dma_start(out=out_ext.ap()[m * 128:(m + 1) * 128, :], in_=o[:])


def _triv_flags(inputs):
    def ones(x):
        return bool(np.all(np.asarray(x) == 1.0))

    def zeros(x):
        return bool(np.all(np.asarray(x) == 0.0))

    return {
        "input_norm": ones(inputs["input_norm_gamma"]) and zeros(inputs["input_norm_beta"]),
        "projector_norm": ones(inputs["projector_norm_gamma"]) and zeros(inputs["projector_norm_beta"]),
        "output_norm": ones(inputs["output_norm_gamma"]) and zeros(inputs["output_norm_beta"]),
        "aggregator_bias": zeros(inputs["aggregator_bias"]),
        "projector_dense1_bias": zeros(inputs["projector_dense1_bias"]),
        "projector_dense2_bias": zeros(inputs["projector_dense2_bias"]),
        "broadcast_bias": zeros(inputs["broadcast_bias"]),
        "gate_bias": zeros(inputs["gate_bias"]),
    }


_GRAPH_CACHE = {}

BF16_INPUTS = ("hidden_states", "aggregator_weight", "projector_dense1_weight",
               "projector_dense2_weight", "broadcast_weight", "gate_weight")


def prep_in_maps(inputs):
    """Build per-core input maps; big tensors are converted to bf16 on the
    host (round-to-nearest) so the NEFF reads half the bytes."""
    import ml_dtypes
    hs = np.ascontiguousarray(
        np.asarray(inputs["hidden_states"], dtype=np.float32).astype(ml_dtypes.bfloat16))
    assert hs.shape == (B, L, D)
    weights = {}
    for n in WEIGHT_NAMES:
        a = np.asarray(inputs[n], dtype=np.float32)
        if n in BF16_INPUTS:
            a = a.astype(ml_dtypes.bfloat16)
        weights[n] = np.ascontiguousarray(a)
    in_maps = []
    for b in range(B):
        m = {"hidden_states": np.ascontiguousarray(hs[b])}
        m.update(weights)
        in_maps.append(m)
    return in_maps


def kernel(**inputs):
    triv = _triv_flags(inputs)
    key = tuple(sorted(triv.items()))
    if key not in _GRAPH_CACHE:
        _GRAPH_CACHE[key] = build_graph(triv)
    nc = _GRAPH_CACHE[key]
    in_maps = prep_in_maps(inputs)
    res = run_bass_kernel_spmd(nc, in_maps, core_ids=list(range(B)))
    out = np.stack([res.results[b]["out"] for b in range(B)], axis=0)
    return out.astype(np.float32)


# revision 45
# speedup vs baseline: 1.2120x; 1.0433x over previous
"""Trainium2 Bass kernel for nn_ContinuousThoughtBlock.

Strategy: pure data-parallel over batch (B=8 -> 8 NeuronCores), zero
collectives.  Each core computes one batch element end-to-end:

  context = mean_L(h)                       [D]
  ctx_n   = LN(context); th0 = ctx_n @ Wagg [D]
  8 paths evolve through 4 residual-MLP steps (bf16 matmuls, weights
  stationary on the PE, activations in a [D-on-partitions, path] layout
  so LayerNorm stats become ones-vector matmuls / partition reductions)
  amps    = pruned softmax over paths (only needed after last step)
  bc      = (sum_p amps_p * th_p) @ Wbc     [D]
  gate    = sigmoid(h @ Wg + bg)            [L, D]  (bf16, spilled to DRAM)
  out     = LN_D(h + gate * bc)             [L, D]

Key mechanics:
  - all f32 weights/activations are loaded with gpsimd casting DMAs
    (f32 DRAM -> bf16 SBUF at line rate; no staging, no cast ops)
  - h is transposed for the gate matmul with SBUF->SBUF DMA-transpose
  - gate spills to DRAM (bf16) and is re-read in the final phase
  - three DMA queues: gpsimd SWDGE = weight/h loads + gate re-read,
    ACT HWDGE = hT transposes + gate spill, SP HWDGE = small vectors +
    output writes
  - LN statistics, softmax/prune, residual adds stay in f32
"""

import numpy as np

import concourse.bass as bass
import concourse.mybir as mybir
import concourse.tile as tile
from concourse import bacc
from concourse.bass_utils import run_bass_kernel_spmd
from concourse.masks import make_identity

# Problem constants (hardcoded per harness contract).
B, L, D, H = 8, 2048, 1024, 4096
NUM_PATHS = 8
NUM_STEPS = 4
PRUNE = 0.1
EPS = 1e-6
KD = D // 128    # 8  D-chunks
KH = H // 128    # 32 H-chunks
ML = L // 128    # 16 L-tiles
INV_SQRT_D = 1.0 / float(np.sqrt(np.float32(D)))

F32 = mybir.dt.float32
BF16 = mybir.dt.bfloat16
AF = mybir.ActivationFunctionType
ALU = mybir.AluOpType
AX = mybir.AxisListType

WEIGHT_NAMES = [
    "input_norm_gamma", "input_norm_beta",
    "aggregator_weight", "aggregator_bias",
    "projector_norm_gamma", "projector_norm_beta",
    "projector_dense1_weight", "projector_dense1_bias",
    "projector_dense2_weight", "projector_dense2_bias",
    "broadcast_weight", "broadcast_bias",
    "gate_weight", "gate_bias",
    "output_norm_gamma", "output_norm_beta",
]


def _bc0(ap, n=128):
    """Broadcast a 1-D AP down n partitions via a stride-0 partition dim."""
    return bass.AP(tensor=ap.tensor, offset=ap.offset, ap=[[0, n]] + list(ap.ap))


def _rep0(ap, n, pos=1):
    """Insert a stride-0 free dim of extent n at position pos."""
    new = list(ap.ap)
    new.insert(pos, [0, n])
    return bass.AP(tensor=ap.tensor, offset=ap.offset, ap=new)


def build_graph(triv, debug=False):
    nc = bacc.Bacc("TRN2", target_bir_lowering=False, debug=False,
                   enable_asserts=True, num_devices=B)

    h_ext = nc.declare_dram_parameter("hidden_states", [L, D], BF16, isOutput=False)
    w_ext = {}
    w_ext["aggregator_weight"] = nc.declare_dram_parameter(
        "aggregator_weight", [D, D], BF16, isOutput=False)
    w_ext["projector_dense1_weight"] = nc.declare_dram_parameter(
        "projector_dense1_weight", [D, H], BF16, isOutput=False)
    w_ext["projector_dense2_weight"] = nc.declare_dram_parameter(
        "projector_dense2_weight", [H, D], BF16, isOutput=False)
    w_ext["broadcast_weight"] = nc.declare_dram_parameter(
        "broadcast_weight", [D, D], BF16, isOutput=False)
    w_ext["gate_weight"] = nc.declare_dram_parameter(
        "gate_weight", [D, D], BF16, isOutput=False)
    for n in ("input_norm_gamma", "input_norm_beta", "aggregator_bias",
              "projector_norm_gamma", "projector_norm_beta",
              "projector_dense1_bias", "projector_dense2_bias",
              "broadcast_bias", "gate_bias",
              "output_norm_gamma", "output_norm_beta"):
        shape = [H] if n == "projector_dense1_bias" else [D]
        w_ext[n] = nc.declare_dram_parameter(n, shape, F32, isOutput=False)
    out_ext = nc.declare_dram_parameter("out", [L, D], F32, isOutput=True)
    dbg = {}
    if debug:
        for nm, shape in (("d_ctxT", [128, KD]), ("d_ctxn", [128, KD]),
                          ("d_th0", [128, KD]), ("d_tT", [128, KD * NUM_PATHS]),
                          ("d_amps", [1, NUM_PATHS]), ("d_bcbf", [128, D]),
                          ("d_gate0", [128, D]), ("d_pre0", [128, D])):
            dbg[nm] = nc.declare_dram_parameter(nm, shape, F32, isOutput=True)

    with tile.TileContext(nc) as tc:
        _build_body(nc, tc, h_ext, w_ext, out_ext, triv, dbg)
    nc.compile()
    return nc


def _dmajor(nc, pool, ps_pool, ident_bf, dram_ap, n, name):
    """DMA a [n*128] DRAM vector into a [128, n] d-major SBUF tile
    (tile[p, k] = v[k*128 + p]) via a bf16 [n,128] load + PE transpose.
    Values are bf16-rounded, acceptable for gamma/beta/bias vectors."""
    rowk = pool.tile([n, 128], BF16, tag="dmaj_rowk")
    nc.gpsimd.dma_start(out=rowk[:], in_=dram_ap.rearrange("(k p) -> k p", p=128))
    ps = ps_pool.tile([128, n], BF16, tag="tr")
    nc.tensor.transpose(ps[:], rowk[:], ident_bf[0:n, 0:n])
    t = pool.tile([128, n], F32, tag=name)
    nc.scalar.copy(t[:], ps[:])
    return t


def _build_body(nc, tc, h_ext, w, out_ext, triv, dbg=None):
    dbg = dbg or {}
    import contextlib
    ctx = contextlib.ExitStack()
    with ctx:
        # ---------------- pools ----------------
        singles = ctx.enter_context(tc.tile_pool(name="singles", bufs=1))
        smalls = ctx.enter_context(tc.tile_pool(name="smalls", bufs=1))
        tstate = ctx.enter_context(tc.tile_pool(name="tstate", bufs=2))
        all_triv = all(triv.values())
        hTm_pool = ctx.enter_context(tc.tile_pool(name="hTm", bufs=2 if all_triv else 1))
        gout = ctx.enter_context(tc.tile_pool(name="gout", bufs=2 if all_triv else 1))
        rows = ctx.enter_context(tc.tile_pool(name="rows", bufs=1))
        wpool = tc.alloc_tile_pool(name="wpool", bufs=1)
        dram = ctx.enter_context(tc.tile_pool(name="dram", bufs=1, space="DRAM"))

        ps_small = ctx.enter_context(tc.tile_pool(name="ps_small", bufs=1, space="PSUM"))
        ps_tr = ctx.enter_context(tc.tile_pool(name="ps_tr", bufs=1, space="PSUM"))
        ps_gate = ctx.enter_context(tc.tile_pool(name="ps_gate", bufs=4, space="PSUM"))
        ps_th = ctx.enter_context(tc.tile_pool(name="ps_th", bufs=2, space="PSUM"))

        # ---------------- constants ----------------
        ident_bf = singles.tile([128, 128], BF16)
        make_identity(nc, ident_bf[:])
        ones_bf = singles.tile([128, 1], BF16)
        nc.vector.memset(ones_bf[:], 1.0)
        ones_f32 = singles.tile([128, 1], F32)
        nc.vector.memset(ones_f32[:], 1.0)
        ones_row = singles.tile([1, 128], F32)
        nc.vector.memset(ones_row[:], 1.0)
        ones_row_bf = singles.tile([1, 128], BF16)
        nc.vector.memset(ones_row_bf[:], 1.0)
        eps1 = singles.tile([1, 1], F32)
        nc.vector.memset(eps1[:], EPS)
        eps_col = singles.tile([128, 1], F32)
        nc.vector.memset(eps_col[:], EPS)

        # resident (bf16) tensors
        h_bf = singles.tile([128, ML, D], BF16)      # 32KB/part
        w1_bf = wpool.tile([128, KD, H], BF16)       # 64KB/part
        w2_bf = wpool.tile([128, KH, D], BF16)       # 64KB/part
        wg_bf = wpool.tile([128, KD, D], BF16)       # 16KB/part
        wab_bf = wpool.tile([128, KD, D], BF16)      # 16KB/part (Wagg, later Wbc)

        gate_dram = dram.tile([L, D], BF16)

        # d-major vectors (only when nontrivial)
        gammaT_in = betaT_in = None
        if not triv["input_norm"]:
            gammaT_in = _dmajor(nc, singles, ps_tr, ident_bf,
                                w["input_norm_gamma"].ap(), KD, "g_in")
            betaT_in = _dmajor(nc, singles, ps_tr, ident_bf,
                               w["input_norm_beta"].ap(), KD, "b_in")
        gammaT_pr = betaT_pr = None
        if not triv["projector_norm"]:
            gammaT_pr = _dmajor(nc, singles, ps_tr, ident_bf,
                                w["projector_norm_gamma"].ap(), KD, "g_pr")
            betaT_pr = _dmajor(nc, singles, ps_tr, ident_bf,
                               w["projector_norm_beta"].ap(), KD, "b_pr")
        baggT = None
        if not triv["aggregator_bias"]:
            baggT = _dmajor(nc, singles, ps_tr, ident_bf,
                            w["aggregator_bias"].ap(), KD, "bagg")
        b1T = None
        if not triv["projector_dense1_bias"]:
            b1T = _dmajor(nc, singles, ps_tr, ident_bf,
                          w["projector_dense1_bias"].ap(), KH, "b1")
        b2T_rep = None
        if not triv["projector_dense2_bias"]:
            b2T = _dmajor(nc, singles, ps_tr, ident_bf,
                          w["projector_dense2_bias"].ap(), KD, "b2")
            b2T_rep = _rep0(b2T[:], NUM_PATHS, pos=2)  # [128, KD, P] view
        gbias_row = None
        if not triv["gate_bias"]:
            gbias_row = rows.tile([1, D], BF16, tag="brow")
            nc.gpsimd.dma_start(out=gbias_row[:],
                                in_=w["gate_bias"].ap().rearrange("(a d) -> a d", a=1))

        # ---------------- phase 1: load h (casting DMA) + context ----------------
        h_src = h_ext.ap().rearrange("(m t p) d -> p m t d", p=128, t=2)
        ctx_ps = ps_small.tile([128, KD], F32, tag="sm")
        for m2 in range(ML // 2):
            nc.sync.dma_start(out=h_bf[:, 2 * m2:2 * m2 + 2, :], in_=h_src[:, m2])
        for k in range(KD):
            for m in range(ML):
                nc.tensor.matmul(ctx_ps[:, k:k + 1],
                                 h_bf[:, m, k * 128:(k + 1) * 128],
                                 ones_bf[:],
                                 start=(m == 0), stop=(m == ML - 1))
        # ctxT[p, k] = context[k*128+p] = mean over L
        ctxT = singles.tile([128, KD], F32)
        nc.scalar.mul(ctxT[:], ctx_ps[:], 1.0 / L)
        if "d_ctxT" in dbg:
            nc.sync.dma_start(out=dbg["d_ctxT"].ap(), in_=ctxT[:])

        # ---------------- phase 2a: Wg (casting DMA) ----------------
        wg_src = w["gate_weight"].ap().rearrange("(k t p) d -> p k t d", p=128, t=2)
        for k2 in range(KD // 2):
            nc.gpsimd.dma_start(out=wg_bf[:, 2 * k2:2 * k2 + 2, :], in_=wg_src[:, k2])

        # ---------------- phase 3: input LN + thought0 ----------------
        sqc = smalls.tile([128, KD], F32, tag="sqc")
        nc.vector.tensor_mul(sqc[:], ctxT[:], ctxT[:])
        cst_ps = ps_small.tile([1, 2 * KD], F32, tag="sm")
        nc.tensor.matmul(cst_ps[0:1, 0:KD], ones_f32[:], ctxT[:], start=True, stop=True)
        nc.tensor.matmul(cst_ps[0:1, KD:2 * KD], ones_f32[:], sqc[:], start=True, stop=True)
        csums = smalls.tile([1, 2], F32, tag="csums")
        nc.vector.tensor_reduce(csums[0:1, 0:1], cst_ps[0:1, 0:KD], axis=AX.X, op=ALU.add)
        nc.vector.tensor_reduce(csums[0:1, 1:2], cst_ps[0:1, KD:2 * KD], axis=AX.X, op=ALU.add)
        cmr = smalls.tile([1, 2], F32, tag="cmr")      # [mean, rstd]
        nc.scalar.mul(cmr[0:1, 0:1], csums[0:1, 0:1], 1.0 / D)
        csq = smalls.tile([1, 2], F32, tag="csq")
        nc.scalar.mul(csq[0:1, 0:1], csums[0:1, 1:2], 1.0 / D)   # E[x^2]
        nc.vector.tensor_mul(csq[0:1, 1:2], cmr[0:1, 0:1], cmr[0:1, 0:1])  # mean^2
        cvar = smalls.tile([1, 1], F32, tag="cvar")
        nc.vector.tensor_sub(cvar[:], csq[0:1, 0:1], csq[0:1, 1:2])
        nc.scalar.activation(cvar[:], cvar[:], AF.Sqrt, bias=eps1[0:1, :])
        nc.vector.reciprocal(cmr[0:1, 1:2], cvar[:])
        cmr_ps = ps_small.tile([128, 2], F32, tag="sm")
        nc.tensor.matmul(cmr_ps[:], ones_row[:], cmr[:], start=True, stop=True)
        cmr_b = smalls.tile([128, 2], F32, tag="cmrb")
        nc.scalar.copy(cmr_b[:], cmr_ps[:])
        ctxn = smalls.tile([128, KD], F32, tag="ctxn")
        nc.vector.tensor_scalar(ctxn[:], ctxT[:], cmr_b[:, 0:1], cmr_b[:, 1:2],
                                op0=ALU.subtract, op1=ALU.mult)
        if gammaT_in is not None:
            nc.vector.tensor_mul(ctxn[:], ctxn[:], gammaT_in[:])
            nc.vector.tensor_add(ctxn[:], ctxn[:], betaT_in[:])
        if "d_ctxn" in dbg:
            nc.sync.dma_start(out=dbg["d_ctxn"].ap(), in_=ctxn[:])
        ctxn_bf = smalls.tile([128, KD], BF16, tag="ctxnbf")
        nc.vector.tensor_copy(ctxn_bf[:], ctxn[:])

        # Wagg (casting DMA into the shared wab buffer)
        wagg_src = w["aggregator_weight"].ap().rearrange("(k t p) d -> p k t d",
                                                         p=128, t=2)
        for k2 in range(KD // 2):
            nc.gpsimd.dma_start(out=wab_bf[:, 2 * k2:2 * k2 + 2, :],
                                in_=wagg_src[:, k2])
        # thought0 = ctx_n @ Wagg, d-major via per-k single-instr psum groups
        th0acc = smalls.tile([128, KD], F32, tag="th0acc")
        for k in range(KD):
            thp = ps_small.tile([128, KD], F32, tag="sm")
            for dm in range(KD):
                nc.tensor.matmul(thp[:, dm:dm + 1],
                                 wab_bf[:, k, dm * 128:(dm + 1) * 128],
                                 ctxn_bf[:, k:k + 1], start=True, stop=True)
            if k == 0:
                nc.vector.tensor_copy(th0acc[:], thp[:])
            else:
                nc.vector.tensor_add(th0acc[:], th0acc[:], thp[:])
        if baggT is not None:
            nc.vector.tensor_add(th0acc[:], th0acc[:], baggT[:])
        if "d_th0" in dbg:
            nc.sync.dma_start(out=dbg["d_th0"].ap(), in_=th0acc[:])
        # seed 8 paths: tT[p, k, q] = th0[k*128+p] * (1 + 0.02 q)
        tT = tstate.tile([128, KD, NUM_PATHS], F32, tag="tT")
        for q in range(NUM_PATHS):
            nc.scalar.mul(tT[:, :, q], th0acc[:], 1.0 + 0.02 * q)

        # ---------------- phase 7: gate matmul (spilled to DRAM) ----------------
        # hT via SBUF->SBUF DMA transpose; lhsT = hT chunks, rhs = Wg.
        # Emitted in slices interleaved with the thought steps so the PE
        # stream stays dense while step inputs arrive.
        def emit_gate_tiles(ms):
            for m in ms:
                hTm = hTm_pool.tile([128, KD, 128], BF16, tag="hTm")
                nc.scalar.dma_start_transpose(out=hTm[:], in_=h_bf[:, m, :])
                for n in range(2):
                    g_ps = ps_gate.tile([128, 512], F32, tag="gps")
                    for k in range(KD):
                        nc.tensor.matmul(g_ps[:], hTm[:, k, :],
                                         wg_bf[:, k, n * 512:(n + 1) * 512],
                                         start=(k == 0),
                                         stop=(k == KD - 1 and gbias_row is None))
                    if gbias_row is not None:
                        nc.tensor.matmul(g_ps[:], ones_row_bf[:],
                                         gbias_row[0:1, n * 512:(n + 1) * 512],
                                         start=False, stop=True)
                    g_sb = gout.tile([128, 512], BF16, tag="gout")
                    nc.scalar.activation(g_sb[:], g_ps[:], AF.Sigmoid)
                    nc.scalar.dma_start(
                        out=gate_dram[m * 128:(m + 1) * 128, n * 512:(n + 1) * 512],
                        in_=g_sb[:])

        emit_gate_tiles(range(0, 6))

        # ---------------- phase 2c/2d: W1 / W2 (casting DMAs) ----------------
        w1_src = w["projector_dense1_weight"].ap().rearrange("(k p) h -> p k h", p=128)
        for k in range(KD):
            nc.sync.dma_start(out=w1_bf[:, k, :], in_=w1_src[:, k])
        w2_src = w["projector_dense2_weight"].ap().rearrange("(k t p) d -> p k t d",
                                                             p=128, t=4)
        for k4 in range(KH // 4):
            nc.gpsimd.dma_start(out=w2_bf[:, 4 * k4:4 * k4 + 4, :], in_=w2_src[:, k4])

        # ---------------- phase 4: thought steps ----------------
        _gate_slices = {0: range(6, 10), 1: range(10, 13), 2: range(13, 16)}
        for step in range(NUM_STEPS):
            if step in _gate_slices:
                emit_gate_tiles(_gate_slices[step])
            last = step == NUM_STEPS - 1
            sq = smalls.tile([128, KD, NUM_PATHS], F32, tag="sq")
            nc.vector.tensor_mul(sq[:], tT[:], tT[:])
            st_ps = ps_small.tile([1, 128], F32, tag="sm")
            nc.tensor.matmul(st_ps[0:1, 0:64], ones_f32[:],
                             tT[:].rearrange("a k q -> a q k"), start=True, stop=True)
            nc.tensor.matmul(st_ps[0:1, 64:128], ones_f32[:],
                             sq[:].rearrange("a k q -> a q k"), start=True, stop=True)
            sums = smalls.tile([1, 2 * NUM_PATHS], F32, tag="sums")
            nc.vector.tensor_reduce(sums[0:1, 0:NUM_PATHS],
                                    st_ps[0:1, 0:64].rearrange("a (q k) -> a q k", k=KD),
                                    axis=AX.X, op=ALU.add)
            nc.vector.tensor_reduce(sums[0:1, NUM_PATHS:],
                                    st_ps[0:1, 64:128].rearrange("a (q k) -> a q k", k=KD),
                                    axis=AX.X, op=ALU.add)
            mr = smalls.tile([1, 2 * NUM_PATHS], F32, tag="mr")  # [mean(8), rstd(8)]
            nc.scalar.mul(mr[0:1, 0:NUM_PATHS], sums[0:1, 0:NUM_PATHS], 1.0 / D)
            msq = smalls.tile([1, NUM_PATHS], F32, tag="msq")
            nc.scalar.mul(msq[0:1, :], sums[0:1, NUM_PATHS:], 1.0 / D)
            m2 = smalls.tile([1, NUM_PATHS], F32, tag="m2")
            nc.vector.tensor_mul(m2[0:1, :], mr[0:1, 0:NUM_PATHS], mr[0:1, 0:NUM_PATHS])
            var = smalls.tile([1, NUM_PATHS], F32, tag="var")
            nc.vector.tensor_sub(var[0:1, :], msq[0:1, :], m2[0:1, :])
            nc.scalar.activation(var[0:1, :], var[0:1, :], AF.Sqrt, bias=eps1[0:1, :])
            nc.vector.reciprocal(mr[0:1, NUM_PATHS:], var[0:1, :])
            mr_ps = ps_small.tile([128, 2 * NUM_PATHS], F32, tag="sm")
            nc.tensor.matmul(mr_ps[:], ones_row[:], mr[:], start=True, stop=True)
            mr_b = smalls.tile([128, 2 * NUM_PATHS], F32, tag="mrb")
            nc.scalar.copy(mr_b[:], mr_ps[:])

            # normalize all (k, q) at once with stride-0 broadcasts
            tn_bf = smalls.tile([128, KD, NUM_PATHS], BF16, tag="tnbf")
            tc_f = smalls.tile([128, KD, NUM_PATHS], F32, tag="tcf")
            nc.vector.tensor_tensor(out=tc_f[:], in0=tT[:],
                                    in1=_rep0(mr_b[:, 0:NUM_PATHS], KD),
                                    op=ALU.subtract)
            if gammaT_pr is not None:
                nc.vector.tensor_tensor(out=tc_f[:], in0=tc_f[:],
                                        in1=_rep0(mr_b[:, NUM_PATHS:], KD),
                                        op=ALU.mult)
                nc.vector.tensor_tensor(out=tc_f[:], in0=tc_f[:],
                                        in1=_rep0(gammaT_pr[:], NUM_PATHS, pos=2),
                                        op=ALU.mult)
                nc.vector.tensor_tensor(out=tn_bf[:], in0=tc_f[:],
                                        in1=_rep0(betaT_pr[:], NUM_PATHS, pos=2),
                                        op=ALU.add)
            else:
                nc.vector.tensor_tensor(out=tn_bf[:], in0=tc_f[:],
                                        in1=_rep0(mr_b[:, NUM_PATHS:], KD),
                                        op=ALU.mult)

            # dense1: x1 = gelu(tn @ W1 [+ b1]) in [H-part, path] layout
            x1_bf = smalls.tile([128, KH // 8, 8, NUM_PATHS], BF16, tag="x1")
            for tblk in range(KH // 8):
                x1_ps = ps_th.tile([128, 8 * NUM_PATHS], F32, tag="th")
                for hs in range(8):
                    mh = tblk * 8 + hs
                    for k in range(KD):
                        nc.tensor.matmul(x1_ps[:, hs * 8:(hs + 1) * 8],
                                         w1_bf[:, k, mh * 128:(mh + 1) * 128],
                                         tn_bf[:, k, :],
                                         start=(k == 0), stop=(k == KD - 1))
                # tanh-gelu (matches jax.nn.gelu approximate=True)
                xs = smalls.tile([128, 8 * NUM_PATHS], F32, tag="gelu_x")
                if b1T is not None:
                    for hs in range(8):
                        mh = tblk * 8 + hs
                        nc.scalar.activation(xs[:, hs * 8:(hs + 1) * 8],
                                             x1_ps[:, hs * 8:(hs + 1) * 8],
                                             AF.Identity, bias=b1T[:, mh:mh + 1])
                else:
                    nc.scalar.copy(xs[:], x1_ps[:])
                u = smalls.tile([128, 8 * NUM_PATHS], F32, tag="gelu_u")
                nc.vector.tensor_mul(u[:], xs[:], xs[:])
                nc.vector.tensor_mul(u[:], u[:], xs[:])
                nc.vector.scalar_tensor_tensor(u[:], u[:], 0.044715, xs[:],
                                               op0=ALU.mult, op1=ALU.add)
                nc.scalar.activation(u[:], u[:], AF.Tanh, scale=0.7978845608028654)
                nc.vector.scalar_tensor_tensor(u[:], u[:], 1.0, xs[:],
                                               op0=ALU.add, op1=ALU.mult)
                nc.scalar.mul(x1_bf[:, tblk].rearrange("a b c -> a (b c)"), u[:], 0.5)

            # dense2 + residual
            y_ps = ps_th.tile([128, KD * NUM_PATHS], F32, tag="th")
            for dm in range(KD):
                for hk in range(KH):
                    nc.tensor.matmul(y_ps[:, dm * 8:(dm + 1) * 8],
                                     w2_bf[:, hk, dm * 128:(dm + 1) * 128],
                                     x1_bf[:, hk // 8, hk % 8, :],
                                     start=(hk == 0), stop=(hk == KH - 1))
            tT_new = tstate.tile([128, KD, NUM_PATHS], F32, tag="tT")
            yv = y_ps[:].rearrange("a (k q) -> a k q", k=KD)
            if b2T_rep is not None:
                nc.vector.tensor_add(tT_new[:], yv, b2T_rep)
                nc.vector.tensor_add(tT_new[:], tT_new[:], tT[:])
            else:
                nc.vector.tensor_add(tT_new[:], yv, tT[:])
            tT = tT_new

            if last:
                sc_ps = ps_small.tile([1, NUM_PATHS], F32, tag="sm")
                for k in range(KD):
                    nc.tensor.matmul(sc_ps[:], ctxT[:, k:k + 1], tT[:, k, :],
                                     start=(k == 0), stop=(k == KD - 1))
                sc = smalls.tile([1, NUM_PATHS], F32, tag="sc")
                nc.scalar.mul(sc[:], sc_ps[:], INV_SQRT_D)
                negmax = smalls.tile([1, 1], F32, tag="negmax")
                nc.vector.tensor_reduce(negmax[:], sc[:], axis=AX.X, op=ALU.max,
                                        negate=True)
                esum = smalls.tile([1, 1], F32, tag="esum")
                ex = smalls.tile([1, NUM_PATHS], F32, tag="ex")
                nc.scalar.activation(ex[:], sc[:], AF.Exp, bias=negmax[0:1, :],
                                     accum_out=esum[:])
                rsum = smalls.tile([1, 1], F32, tag="rsum")
                nc.vector.reciprocal(rsum[:], esum[:])
                amps0 = smalls.tile([1, NUM_PATHS], F32, tag="amps0")
                nc.vector.tensor_scalar(amps0[:], ex[:], rsum[0:1, :], None, op0=ALU.mult)
                mask = smalls.tile([1, NUM_PATHS], F32, tag="mask")
                nc.vector.tensor_scalar(mask[:], amps0[:], PRUNE, None, op0=ALU.is_ge)
                pruned = smalls.tile([1, NUM_PATHS], F32, tag="pruned")
                nc.vector.tensor_mul(pruned[:], amps0[:], mask[:])
                psum_s = smalls.tile([1, 1], F32, tag="psums")
                nc.vector.tensor_reduce(psum_s[:], pruned[:], axis=AX.X, op=ALU.add)
                nc.vector.tensor_scalar(psum_s[:], psum_s[:], EPS, None, op0=ALU.add)
                rr = smalls.tile([1, 1], F32, tag="rr")
                nc.vector.reciprocal(rr[:], psum_s[:])
                ampsF = smalls.tile([1, NUM_PATHS], F32, tag="ampsF")
                nc.vector.tensor_scalar(ampsF[:], pruned[:], rr[0:1, :], None, op0=ALU.mult)

        if "d_tT" in dbg:
            nc.sync.dma_start(out=dbg["d_tT"].ap(),
                              in_=tT[:].rearrange("a k q -> a (k q)"))
        if "d_amps" in dbg:
            nc.sync.dma_start(out=dbg["d_amps"].ap(), in_=ampsF[:])

        # ---------------- phase 5: collapse + bc ----------------
        ab_ps = ps_small.tile([128, NUM_PATHS], F32, tag="sm")
        nc.tensor.matmul(ab_ps[:], ones_row[:], ampsF[:], start=True, stop=True)
        amps_sb = smalls.tile([128, NUM_PATHS], F32, tag="ampssb")
        nc.scalar.copy(amps_sb[:], ab_ps[:])
        prod = smalls.tile([128, KD, NUM_PATHS], F32, tag="prod")
        nc.vector.tensor_tensor(out=prod[:], in0=tT[:], in1=_rep0(amps_sb[:], KD),
                                op=ALU.mult)
        finalT = smalls.tile([128, KD], F32, tag="finalT")
        nc.vector.tensor_reduce(finalT[:], prod[:], axis=AX.X, op=ALU.add)
        finalT_bf = smalls.tile([128, KD], BF16, tag="finalTbf")
        nc.vector.tensor_copy(finalT_bf[:], finalT[:])

        # Wbc overwrites the shared wab buffer (casting DMA)
        wbc_src = w["broadcast_weight"].ap().rearrange("(k t p) d -> p k t d",
                                                       p=128, t=2)
        for k2 in range(KD // 2):
            nc.gpsimd.dma_start(out=wab_bf[:, 2 * k2:2 * k2 + 2, :],
                                in_=wbc_src[:, k2])
        # bc row [1, D] in two 512-halves, then K=1 matmul broadcast
        bc_bf = singles.tile([128, D], BF16)
        for n in range(2):
            bc_ps = ps_gate.tile([1, 512], F32, tag="gps")
            for k in range(KD):
                nc.tensor.matmul(bc_ps[:],
                                 finalT_bf[:, k:k + 1],
                                 wab_bf[:, k, n * 512:(n + 1) * 512],
                                 start=(k == 0), stop=(k == KD - 1))
            bc_half = rows.tile([1, 512], F32, tag="row")
            if not triv["broadcast_bias"]:
                bbh = rows.tile([1, 512], F32, tag="brow")
                nc.sync.dma_start(
                    out=bbh[:],
                    in_=w["broadcast_bias"].ap()[n * 512:(n + 1) * 512]
                        .rearrange("(a d) -> a d", a=1))
                nc.vector.tensor_add(bc_half[:], bc_ps[:], bbh[:])
            else:
                nc.scalar.copy(bc_half[:], bc_ps[:])
            bcb_ps = ps_gate.tile([128, 512], F32, tag="gps")
            nc.tensor.matmul(bcb_ps[:], ones_row[:], bc_half[:], start=True, stop=True)
            nc.scalar.copy(bc_bf[:, n * 512:(n + 1) * 512], bcb_ps[:])
        if "d_bcbf" in dbg:
            nc.sync.dma_start(out=dbg["d_bcbf"].ap(), in_=bc_bf[:])

        # release the weight pool; final-phase pools reuse the space
        wpool.release()
        gin = ctx.enter_context(tc.tile_pool(name="gin", bufs=4))
        fin = ctx.enter_context(tc.tile_pool(name="fin", bufs=4))
        gamma_out_b = beta_out_b = None
        if not triv["output_norm"]:
            fin1 = ctx.enter_context(tc.tile_pool(name="fin1", bufs=1))
            gamma_out_b = fin1.tile([128, D], F32)
            nc.sync.dma_start(out=gamma_out_b[:], in_=_bc0(w["output_norm_gamma"].ap()))
            beta_out_b = fin1.tile([128, D], F32)
            nc.sync.dma_start(out=beta_out_b[:], in_=_bc0(w["output_norm_beta"].ap()))

        # ---------------- phase 8: final LN + output ----------------
        # all elementwise work in bf16 (DVE 4x mode); stats accumulate in f32
        for m in range(ML):
            g_in = gin.tile([128, D], BF16, tag="gin")
            nc.gpsimd.dma_start(out=g_in[:], in_=gate_dram[m * 128:(m + 1) * 128, :])
            if m == 0 and "d_gate0" in dbg:
                nc.sync.dma_start(out=dbg["d_gate0"].ap(), in_=g_in[:])
            p1 = gin.tile([128, D], BF16, tag="p1")
            nc.vector.tensor_mul(p1[:], g_in[:], bc_bf[:])
            # pre = p1 + h, with fused row-sum accumulation
            pre = fin.tile([128, D], BF16, tag="pre")
            rs = fin.tile([128, 2], F32, tag="rs")   # [rowsum, rowsumsq]
            nc.vector.scalar_tensor_tensor(pre[:], p1[:], 1.0, h_bf[:, m, :],
                                           op0=ALU.mult, op1=ALU.add,
                                           accum_out=rs[:, 0:1])
            if m == 0 and "d_pre0" in dbg:
                nc.sync.dma_start(out=dbg["d_pre0"].ap(), in_=pre[:])
            sqs = fin.tile([128, D], BF16, tag="sqs")
            nc.scalar.activation(sqs[:], pre[:], AF.Square, accum_out=rs[:, 1:2])
            mv = fin.tile([128, 2], F32, tag="mv")   # [mean, E[x^2]]
            nc.vector.tensor_scalar(mv[:], rs[:], 1.0 / D, None, op0=ALU.mult)
            var = fin.tile([128, 1], F32, tag="var")
            nc.vector.tensor_tensor(out=var[:], in0=mv[:, 0:1], in1=mv[:, 0:1],
                                    op=ALU.mult)
            nc.vector.tensor_sub(var[:], mv[:, 1:2], var[:])
            sd = fin.tile([128, 1], F32, tag="sd")
            nc.scalar.activation(sd[:], var[:], AF.Sqrt, bias=eps_col[:])
            rstd = fin.tile([128, 1], F32, tag="rstd")
            nc.vector.reciprocal(rstd[:], sd[:])
            o = fin.tile([128, D], F32, tag="o")
            nc.vector.tensor_scalar(o[:], pre[:], mv[:, 0:1], rstd[:, 0:1],
                                    op0=ALU.subtract, op1=ALU.mult)
            if gamma_out_b is not None:
                nc.vector.tensor_mul(o[:], o[:], gamma_out_b[:])
                nc.vector.tensor_add(o[:], o[:], beta_out_b[:])
            nc.sync.dma_start(out=out_ext.ap()[m * 128:(m + 1) * 128, :], in_=o[:])


def _triv_flags(inputs):
    def ones(x):
        return bool(np.all(np.asarray(x) == 1.0))

    def zeros(x):
        return bool(np.all(np.asarray(x) == 0.0))

    return {
        "input_norm": ones(inputs["input_norm_gamma"]) and zeros(inputs["input_norm_beta"]),
        "projector_norm": ones(inputs["projector_norm_gamma"]) and zeros(inputs["projector_norm_beta"]),
        "output_norm": ones(inputs["output_norm_gamma"]) and zeros(inputs["output_norm_beta"]),
        "aggregator_bias": zeros(inputs["aggregator_bias"]),
        "projector_dense1_bias": zeros(inputs["projector_dense1_bias"]),
        "projector_dense2_bias": zeros(inputs["projector_dense2_bias"]),
        "broadcast_bias": zeros(inputs["broadcast_bias"]),
        "gate_bias": zeros(inputs["gate_bias"]),
    }


_GRAPH_CACHE = {}

BF16_INPUTS = ("hidden_states", "aggregator_weight", "projector_dense1_weight",
               "projector_dense2_weight", "broadcast_weight", "gate_weight")


def prep_in_maps(inputs):
    """Build per-core input maps; big tensors are converted to bf16 on the
    host (round-to-nearest) so the NEFF reads half the bytes."""
    import ml_dtypes
    hs = np.ascontiguousarray(
        np.asarray(inputs["hidden_states"], dtype=np.float32).astype(ml_dtypes.bfloat16))
    assert hs.shape == (B, L, D)
    weights = {}
    for n in WEIGHT_NAMES:
        a = np.asarray(inputs[n], dtype=np.float32)
        if n in BF16_INPUTS:
            a = a.astype(ml_dtypes.bfloat16)
        weights[n] = np.ascontiguousarray(a)
    in_maps = []
    for b in range(B):
        m = {"hidden_states": np.ascontiguousarray(hs[b])}
        m.update(weights)
        in_maps.append(m)
    return in_maps


def kernel(**inputs):
    triv = _triv_flags(inputs)
    key = tuple(sorted(triv.items()))
    if key not in _GRAPH_CACHE:
        _GRAPH_CACHE[key] = build_graph(triv)
    nc = _GRAPH_CACHE[key]
    in_maps = prep_in_maps(inputs)
    res = run_bass_kernel_spmd(nc, in_maps, core_ids=list(range(B)))
    out = np.stack([res.results[b]["out"] for b in range(B)], axis=0)
    return out.astype(np.float32)
